# revision 1
# baseline (speedup 1.0000x reference)
"""Causal self-attention (RoPE + QK-RMSNorm, GQA 16q/8kv) Trainium2 Bass kernel.

Sharding: 8 cores = 2 batch x 4 tensor-parallel. Core c handles batch b=c//4 and
q-heads [4*tp, 4*tp+4), kv-heads [2*tp, 2*tp+2) where tp=c%4. Each core returns a
partial (T, C) output = O_heads @ wo[rows of its heads]; host sums the 4 partials
per batch (the "all-reduce after c_proj").

Matmuls run in bf16 (fp32 PSUM accumulation); softmax row-sum normalization and
RMS statistics stay in fp32/fp32r.
"""
import sys
import math

sys.path.insert(0, "/opt/trn_rl_repo")

import numpy as np
import ml_dtypes
import concourse.bacc as bacc
import concourse.mybir as mybir
import concourse.tile as tile
from concourse.bass_utils import run_bass_kernel_spmd

P = 128
T = 2048
C = 2048
KO = C // P          # 16 contraction tiles
D = 128              # head dim
NQ = 4               # q heads per core
NK = 2               # kv heads per core
NF = NQ + NK         # 6 rope/rms feature blocks (4 q + 2 k)
FQ = NQ * D          # 512
FK = NK * D          # 256
TCH = 512            # phase-1 T-chunk
NCHUNK = T // TCH    # 4
SPAN = 512           # attention q-span
NSPAN = T // SPAN    # 4
KB = T // P          # 16 key blocks
SCALE = 1.0 / math.sqrt(D)
EPS = 1.1920929e-07

f32 = mybir.dt.float32
f32r = mybir.dt.float32r
bf16 = mybir.dt.bfloat16

AF = mybir.ActivationFunctionType


def build():
    nc = bacc.Bacc("TRN2", target_bir_lowering=False)
    xT = nc.dram_tensor("xT", (C, T), bf16, kind="ExternalInput")
    wq = nc.dram_tensor("wq", (C, FQ), bf16, kind="ExternalInput")
    wk = nc.dram_tensor("wk", (C, FK), bf16, kind="ExternalInput")
    wv = nc.dram_tensor("wv", (C, FK), bf16, kind="ExternalInput")
    wo = nc.dram_tensor("wo", (FQ, C), bf16, kind="ExternalInput")
    cc = nc.dram_tensor("cc", (P, T), f32, kind="ExternalInput")    # [cos; cos]
    ss = nc.dram_tensor("ss", (P, T), f32, kind="ExternalInput")    # [sin; -sin]
    maskT = nc.dram_tensor("maskT", (P, 4, SPAN), bf16, kind="ExternalInput")
    ident = nc.dram_tensor("ident", (P, P), bf16, kind="ExternalInput")
    y = nc.dram_tensor("y", (T, C), f32, kind="ExternalOutput")

    xT_r = xT.rearrange("(ko p) t -> p ko t", p=P)
    wq_r = wq.rearrange("(ko p) f -> p ko f", p=P)
    wk_r = wk.rearrange("(ko p) f -> p ko f", p=P)
    wv_r = wv.rearrange("(ko p) f -> p ko f", p=P)
    wo_r = wo.rearrange("(ko p) n -> p ko n", p=P)

    with tile.TileContext(nc) as tc:
        with tc.tile_pool(name="persist", bufs=1) as persist:
            # persistent across phases
            qk_rt = persist.tile([P, NF, T], bf16, tag="qk_rt")   # roped+normed qT/kT
            v_sb = persist.tile([P, KB, FK], bf16, tag="v_sb")    # V natural [t-part, kb, feat]
            cc_sb = persist.tile([P, T], f32, tag="cc_sb")
            ss_sb = persist.tile([P, T], f32, tag="ss_sb")
            id_sb = persist.tile([P, P], bf16, tag="id_sb")
            ones_col = persist.tile([P, 1], bf16, tag="ones_col")    # sums lhsT
            ones_row = persist.tile([1, P], f32r, tag="ones_row")    # bcast lhsT
            eps_sb = persist.tile([P, 1], f32, tag="eps_sb")
            zero_sb = persist.tile([1, 1], f32, tag="zero_sb")
            nc.vector.memset(zero_sb[:], 0.0)
            ones_f32 = persist.tile([P, 1], f32, tag="ones_f32")
            ones_row_f32 = persist.tile([1, P], f32, tag="ones_row_f32")
            nc.sync.dma_start(cc_sb[:], cc[:, :])
            nc.sync.dma_start(ss_sb[:], ss[:, :])
            nc.sync.dma_start(id_sb[:], ident[:, :])
            nc.vector.memset(eps_sb[:], EPS)
            nc.vector.memset(ones_f32[:], 1.0)
            nc.vector.memset(ones_row_f32[:], 1.0)
            nc.vector.tensor_copy(ones_col[:], ones_f32[:])
            nc.vector.tensor_copy(ones_row[:], ones_row_f32[:])

            # ------- Phase 1: QKV projections + RoPE + RMS norm + V transpose -------
            with (
                tc.tile_pool(name="ph1w", bufs=1) as wpool,
                tc.tile_pool(name="ph1x", bufs=2) as xpool,
                tc.tile_pool(name="ph1t", bufs=3) as tpool,
                tc.tile_pool(name="ph1ps", bufs=3, space="PSUM") as ps1,
                tc.tile_pool(name="ph1tr", bufs=1, space="PSUM") as pstr,
                tc.tile_pool(name="ph1ms", bufs=2, space="PSUM") as psms,
                tc.tile_pool(name="ph1rb", bufs=2, space="PSUM") as psrb,
            ):
                wq_sb = wpool.tile([P, KO, FQ], bf16, tag="wq_sb")
                wk_sb = wpool.tile([P, KO, FK], bf16, tag="wk_sb")
                wv_sb = wpool.tile([P, KO, FK], bf16, tag="wv_sb")
                nc.sync.dma_start(wq_sb[:], wq_r)
                nc.sync.dma_start(wk_sb[:], wk_r)
                nc.sync.dma_start(wv_sb[:], wv_r)

                for tch in range(NCHUNK):
                    t0 = tch * TCH
                    xt = xpool.tile([P, KO, TCH], bf16, tag="xt")
                    # per-ko DMAs so matmuls can start as slices land
                    for ko in range(KO):
                        nc.sync.dma_start(xt[:, ko, :], xT_r[:, ko, t0 : t0 + TCH])
                    # qT / kT feature blocks (4 q heads + 2 k heads)
                    sqs = []
                    for fb in range(NF):
                        if fb < NQ:
                            w_ap = wq_sb[:, :, fb * D : (fb + 1) * D]
                        else:
                            w_ap = wk_sb[:, :, (fb - NQ) * D : (fb - NQ + 1) * D]
                        pqk = ps1.tile([P, TCH], f32, tag="ps_qkv")
                        for ko in range(KO):
                            nc.tensor.matmul(
                                pqk[:], w_ap[:, ko], xt[:, ko, :],
                                start=(ko == 0), stop=(ko == KO - 1),
                            )
                        # rope: raw chunk + half-swapped chunk (fp32), write bf16
                        raw = tpool.tile([P, TCH], f32, tag="rope_raw")
                        nc.vector.tensor_copy(raw[:], pqk[:])
                        swp = tpool.tile([P, TCH], f32, tag="rope_swp")
                        nc.sync.dma_start(swp[0:64, :], raw[64:128, :])
                        nc.sync.dma_start(swp[64:128, :], raw[0:64, :])
                        tmpa = tpool.tile([P, TCH], f32, tag="rope_tmpa")
                        tmpb = tpool.tile([P, TCH], f32, tag="rope_tmpb")
                        seg = qk_rt[:, fb, t0 : t0 + TCH]
                        nc.vector.tensor_mul(tmpa[:], raw[:], cc_sb[:, t0 : t0 + TCH])
                        nc.vector.tensor_mul(tmpb[:], swp[:], ss_sb[:, t0 : t0 + TCH])
                        nc.vector.tensor_add(seg, tmpa[:], tmpb[:])
                        # RMS stats: sum of squares over head dim (partitions)
                        sq = tpool.tile([P, TCH], bf16, tag="sq")
                        nc.vector.tensor_mul(sq[:], seg, seg)
                        pms = psms.tile([1, TCH], f32, tag="ps_ms")
                        nc.tensor.matmul(pms[:], ones_col[:], sq[:], start=True, stop=True)
                        # rstd = exp(-0.5 * ln(ms/D + eps)) — both on ACT, off the PE path
                        lnms = tpool.tile([1, TCH], f32, tag="lnms")
                        nc.scalar.activation(
                            lnms[:], pms[:], AF.Ln, bias=eps_sb[0:1, :], scale=1.0 / D
                        )
                        rstd = tpool.tile([1, TCH], f32r, tag="rstd")
                        nc.scalar.activation(rstd[:], lnms[:], AF.Exp, scale=-0.5)
                        sqs.append((seg, rstd))
                    # RMS apply pass — bcast matmuls run a full block later so the
                    # ACT chain has drained and the PE never head-of-line blocks
                    for seg, rstd in sqs:
                        pb = psrb.tile([P, TCH], f32, tag="ps_b")
                        nc.tensor.matmul(pb[:], ones_row[:], rstd[:], start=True, stop=True)
                        nc.vector.tensor_mul(seg, seg, pb[:])
                    # vT blocks -> transpose -> V natural
                    for vfb in range(NK):
                        w_ap = wv_sb[:, :, vfb * D : (vfb + 1) * D]
                        pvt = ps1.tile([P, TCH], f32, tag="ps_qkv")
                        for ko in range(KO):
                            nc.tensor.matmul(
                                pvt[:], w_ap[:, ko], xt[:, ko, :],
                                start=(ko == 0), stop=(ko == KO - 1),
                            )
                        vt_sb = tpool.tile([P, TCH], bf16, tag="vt_sb")
                        nc.vector.tensor_copy(vt_sb[:], pvt[:])
                        for tb in range(TCH // P):
                            ptr = pstr.tile([P, P], bf16, tag="ps_tr")
                            nc.tensor.transpose(
                                ptr[:], vt_sb[:, tb * P : (tb + 1) * P], id_sb[:]
                            )
                            nc.vector.tensor_copy(
                                v_sb[:, tch * (TCH // P) + tb, vfb * D : (vfb + 1) * D],
                                ptr[:],
                            )

            # ---------------- Phase 3: attention + Phase 4: output projection ------------
            with (
                tc.tile_pool(name="ph3s", bufs=1) as p3s,
                tc.tile_pool(name="ph3t", bufs=6) as p3,
            ):
                ot_sb = p3s.tile([P, NQ, T], bf16, tag="ot_sb")
                mask_sb = p3s.tile([P, 4, SPAN], bf16, tag="mask_sb")
                wo_sb = p3s.tile([P, NQ, C], bf16, tag="wo_sb")
                nc.sync.dma_start(mask_sb[:], maskT[:, :, :])
                nc.sync.dma_start(wo_sb[:], wo_r)

                with (
                    tc.tile_pool(name="ph3ps", bufs=3, space="PSUM") as ps3,
                    tc.tile_pool(name="ph3ot", bufs=2, space="PSUM") as psot,
                    tc.tile_pool(name="ph3m", bufs=1, space="PSUM") as psm,
                ):
                  for s in range(NSPAN):
                    q0 = s * SPAN
                    nkb = 4 * s + 4
                    for h in range(NQ):
                        j = h // 2
                        ot_ps = psot.tile([P, SPAN], f32, tag="ot_ps")
                        sum_ps = psot.tile([1, SPAN], f32, tag="sum_ps")
                        q_ap = qk_rt[:, h, q0 : q0 + SPAN]
                        for kb in range(nkb):
                            st_ps = ps3.tile([P, SPAN], f32, tag="st_ps")
                            nc.tensor.matmul(
                                st_ps[:],
                                qk_rt[:, NQ + j, kb * P : (kb + 1) * P],
                                q_ap,
                                start=True, stop=True,
                            )
                            pt = p3.tile([P, SPAN], bf16, tag="pt")
                            nc.scalar.activation(pt[:], st_ps[:], AF.Exp, scale=SCALE)
                            if kb >= 4 * s:
                                nc.vector.tensor_mul(
                                    pt[:], pt[:], mask_sb[:, kb - 4 * s, :]
                                )
                            nc.tensor.matmul(
                                ot_ps[:],
                                v_sb[:, kb, j * D : (j + 1) * D],
                                pt[:],
                                start=(kb == 0), stop=(kb == nkb - 1),
                                skip_group_check=True,
                            )
                            nc.tensor.matmul(
                                sum_ps[:],
                                ones_col[:],
                                pt[:],
                                start=(kb == 0), stop=(kb == nkb - 1),
                                skip_group_check=True,
                            )
                        # normalization: 1/sums = exp(-ln(sums)) on ACT, then bcast
                        lns = p3.tile([1, SPAN], f32, tag="lns")
                        nc.scalar.activation(lns[:], sum_ps[:], AF.Ln)
                        rec = p3.tile([1, SPAN], f32r, tag="rec")
                        nc.scalar.activation(rec[:], lns[:], AF.Exp, scale=-1.0)
                        bc_ps = psm.tile([P, SPAN], f32, tag="m512")
                        nc.tensor.matmul(bc_ps[:], ones_row[:], rec[:], start=True, stop=True)
                        bc_sb = p3.tile([P, SPAN], f32, tag="bc_sb")
                        nc.scalar.activation(bc_sb[:], bc_ps[:], AF.Copy)
                        nc.vector.tensor_mul(
                            ot_sb[:, h, q0 : q0 + SPAN], ot_ps[:], bc_sb[:]
                        )

                    # output projection for the T-blocks of this span
                    for tb in range(4 * s, 4 * s + 4):
                        for nch in range(C // 512):
                            yps = psm.tile([P, 512], f32, tag="m512")
                            for h in range(NQ):
                                nc.tensor.matmul(
                                    yps[:],
                                    ot_sb[:, h, tb * P : (tb + 1) * P],
                                    wo_sb[:, h, nch * 512 : (nch + 1) * 512],
                                    start=(h == 0), stop=(h == NQ - 1),
                                )
                            ysb = p3.tile([P, 512], f32, tag="ysb")
                            nc.vector.tensor_copy(ysb[:], yps[:])
                            nc.sync.dma_start(
                                y[tb * P : (tb + 1) * P, nch * 512 : (nch + 1) * 512],
                                ysb[:],
                            )
    nc.compile()
    return nc


_NC_CACHE = None


def _get_nc():
    global _NC_CACHE
    if _NC_CACHE is None:
        _NC_CACHE = build()
    return _NC_CACHE


def _host_inputs(x, cos, sin, wq, wk, wv, wo):
    """Build the 8 per-core input maps."""
    bft = ml_dtypes.bfloat16
    cosT = np.ascontiguousarray(cos[0, :, 0, :].T).astype(np.float32)  # (64, T)
    sinT = np.ascontiguousarray(sin[0, :, 0, :].T).astype(np.float32)
    cc = np.concatenate([cosT, cosT], axis=0)          # (128, T)
    ss = np.concatenate([sinT, -sinT], axis=0)
    # maskT[r][k, q] = 1 if q >= 128*r + k  (within a 512-q span, k-block offset r)
    qidx = np.arange(SPAN)[None, None, :]
    kidx = np.arange(P)[:, None, None]
    ridx = np.arange(4)[None, :, None]
    maskT = (qidx >= P * ridx + kidx).astype(bft)  # (128, 4, 512)
    ident = np.eye(P, dtype=np.float32).astype(bft)

    xTs = [np.ascontiguousarray(x[b].T).astype(bft) for b in range(2)]
    wq16 = wq.astype(bft)
    wk16 = wk.astype(bft)
    wv16 = wv.astype(bft)
    wo16 = wo.astype(bft)
    in_maps = []
    for c in range(8):
        b, tp = divmod(c, 4)
        in_maps.append(
            {
                "xT": xTs[b],
                "wq": np.ascontiguousarray(wq16[:, tp * FQ : (tp + 1) * FQ]),
                "wk": np.ascontiguousarray(wk16[:, tp * FK : (tp + 1) * FK]),
                "wv": np.ascontiguousarray(wv16[:, tp * FK : (tp + 1) * FK]),
                "wo": np.ascontiguousarray(wo16[tp * FQ : (tp + 1) * FQ, :]),
                "cc": cc,
                "ss": ss,
                "maskT": maskT,
                "ident": ident,
            }
        )
    return in_maps


def kernel(x, cos, sin, wq, wk, wv, wo, trace=False):
    x = np.asarray(x, dtype=np.float32)
    cos = np.asarray(cos, dtype=np.float32)
    sin = np.asarray(sin, dtype=np.float32)
    wq = np.asarray(wq, dtype=np.float32)
    wk = np.asarray(wk, dtype=np.float32)
    wv = np.asarray(wv, dtype=np.float32)
    wo = np.asarray(wo, dtype=np.float32)

    nc = _get_nc()
    in_maps = _host_inputs(x, cos, sin, wq, wk, wv, wo)
    res = run_bass_kernel_spmd(nc, in_maps, core_ids=list(range(8)), trace=trace)
    out = np.zeros((2, T, C), dtype=np.float32)
    for c in range(8):
        b = c // 4
        out[b] += res.results[c]["y"]
    if trace:
        return out, res
    return out



# revision 3
# speedup vs baseline: 1.0594x; 1.0594x over previous
"""Causal self-attention (RoPE + QK-RMSNorm, GQA 16q/8kv) Trainium2 Bass kernel.

Sharding: 8 cores = 2 batch x 4 tensor-parallel. Core c handles batch b=c//4 and
q-heads [4*tp, 4*tp+4), kv-heads [2*tp, 2*tp+2) where tp=c%4. Each core returns a
partial (T, C) output = O_heads @ wo[rows of its heads]; host sums the 4 partials
per batch (the "all-reduce after c_proj").

Matmuls run in bf16 (fp32 PSUM accumulation); softmax row-sum normalization and
RMS statistics stay in fp32/fp32r.

Phase-3 structure: scores for 2 key-blocks land in one 2-bank PSUM tile, one exp
instruction covers both; row-sums use a DVE pair-add + one accumulating matmul
per group; projection matmuls of the previous span interleave into the current
span's attention stream to keep the PE dense (HAM stays at full clock).
"""
import sys
import math

sys.path.insert(0, "/opt/trn_rl_repo")

import numpy as np
import ml_dtypes
import concourse.bacc as bacc
import concourse.mybir as mybir
import concourse.tile as tile
from concourse.bass_utils import run_bass_kernel_spmd

P = 128
T = 2048
C = 2048
KO = C // P          # 16 contraction tiles
D = 128              # head dim
NQ = 4               # q heads per core
NK = 2               # kv heads per core
NF = NQ + NK         # 6 rope/rms feature blocks (4 q + 2 k)
FQ = NQ * D          # 512
FK = NK * D          # 256
TCH = 512            # phase-1 T-chunk
NCHUNK = T // TCH    # 4
SPAN = 512           # attention q-span
NSPAN = T // SPAN    # 4
KB = T // P          # 16 key blocks
SCALE = 1.0 / math.sqrt(D)
EPS = 1.1920929e-07

f32 = mybir.dt.float32
f32r = mybir.dt.float32r
bf16 = mybir.dt.bfloat16

AF = mybir.ActivationFunctionType


def build():
    nc = bacc.Bacc("TRN2", target_bir_lowering=False)
    xT = nc.dram_tensor("xT", (C, T), bf16, kind="ExternalInput")
    wq = nc.dram_tensor("wq", (C, FQ), bf16, kind="ExternalInput")
    wk = nc.dram_tensor("wk", (C, FK), bf16, kind="ExternalInput")
    wv = nc.dram_tensor("wv", (C, FK), bf16, kind="ExternalInput")
    wo = nc.dram_tensor("wo", (FQ, C), bf16, kind="ExternalInput")
    cc = nc.dram_tensor("cc", (P, T), bf16, kind="ExternalInput")    # [cos; cos]
    ss = nc.dram_tensor("ss", (P, T), bf16, kind="ExternalInput")    # [sin; -sin]
    maskT = nc.dram_tensor("maskT", (P, 4, SPAN), bf16, kind="ExternalInput")
    ident = nc.dram_tensor("ident", (P, P), bf16, kind="ExternalInput")
    y = nc.dram_tensor("y", (T, C), bf16, kind="ExternalOutput")

    xT_r = xT.rearrange("(ko p) t -> p ko t", p=P)
    wq_r = wq.rearrange("(ko p) f -> p ko f", p=P)
    wk_r = wk.rearrange("(ko p) f -> p ko f", p=P)
    wv_r = wv.rearrange("(ko p) f -> p ko f", p=P)
    wo_r = wo.rearrange("(ko p) n -> p ko n", p=P)

    with tile.TileContext(nc) as tc:
        with tc.tile_pool(name="persist", bufs=1) as persist:
            # persistent across phases
            qk_rt = persist.tile([P, NF, T], bf16, tag="qk_rt")   # roped+normed qT/kT
            v_sb = persist.tile([P, KB, FK], bf16, tag="v_sb")    # V natural [t-part, kb, feat]
            cc_sb = persist.tile([P, T], bf16, tag="cc_sb")
            ss_sb = persist.tile([P, T], bf16, tag="ss_sb")
            id_sb = persist.tile([P, P], bf16, tag="id_sb")
            ones_col = persist.tile([P, 1], bf16, tag="ones_col")    # sums lhsT
            ones_row = persist.tile([1, P], f32r, tag="ones_row")    # bcast lhsT
            eps_sb = persist.tile([P, 1], f32, tag="eps_sb")
            zero_sb = persist.tile([1, 1], f32, tag="zero_sb")
            nc.vector.memset(zero_sb[:], 0.0)
            ones_f32 = persist.tile([P, 1], f32, tag="ones_f32")
            ones_row_f32 = persist.tile([1, P], f32, tag="ones_row_f32")
            nc.sync.dma_start(cc_sb[:], cc[:, :])
            nc.sync.dma_start(ss_sb[:], ss[:, :])
            nc.sync.dma_start(id_sb[:], ident[:, :])
            nc.vector.memset(eps_sb[:], EPS)
            nc.vector.memset(ones_f32[:], 1.0)
            nc.vector.memset(ones_row_f32[:], 1.0)
            nc.vector.tensor_copy(ones_col[:], ones_f32[:])
            nc.vector.tensor_copy(ones_row[:], ones_row_f32[:])

            # ------- Phase 1: QKV projections + RoPE + RMS norm + V transpose -------
            with (
                tc.tile_pool(name="ph1w", bufs=1) as wpool,
                tc.tile_pool(name="ph1x", bufs=2) as xpool,
                tc.tile_pool(name="ph1t", bufs=3) as tpool,
                tc.tile_pool(name="ph1ps", bufs=2, space="PSUM") as ps1,
                tc.tile_pool(name="ph1tr", bufs=2, space="PSUM") as pstr,
                tc.tile_pool(name="ph1ms", bufs=1, space="PSUM") as psms,
                tc.tile_pool(name="ph1rb", bufs=3, space="PSUM") as psrb,
            ):
                wq_sb = wpool.tile([P, KO, FQ], bf16, tag="wq_sb")
                wk_sb = wpool.tile([P, KO, FK], bf16, tag="wk_sb")
                wv_sb = wpool.tile([P, KO, FK], bf16, tag="wv_sb")
                # per-ko weight DMAs so the first matmuls can start early
                for ko in range(KO):
                    nc.sync.dma_start(wq_sb[:, ko, :], wq_r[:, ko, :])
                    nc.sync.dma_start(wk_sb[:, ko, :], wk_r[:, ko, :])
                    nc.sync.dma_start(wv_sb[:, ko, :], wv_r[:, ko, :])

                for tch in range(NCHUNK):
                    t0 = tch * TCH
                    xt = xpool.tile([P, KO, TCH], bf16, tag="xt")
                    # per-ko DMAs so matmuls can start as slices land
                    for ko in range(KO):
                        nc.sync.dma_start(xt[:, ko, :], xT_r[:, ko, t0 : t0 + TCH])
                    # qT / kT feature blocks (4 q heads + 2 k heads)
                    sqs = []
                    for fb in range(NF):
                        if fb < NQ:
                            w_ap = wq_sb[:, :, fb * D : (fb + 1) * D]
                        else:
                            w_ap = wk_sb[:, :, (fb - NQ) * D : (fb - NQ + 1) * D]
                        pqk = ps1.tile([P, TCH], f32, tag="ps_qkv")
                        for ko in range(KO):
                            nc.tensor.matmul(
                                pqk[:], w_ap[:, ko], xt[:, ko, :],
                                start=(ko == 0), stop=(ko == KO - 1),
                            )
                        # rope: raw chunk + half-swapped chunk (bf16), 2x DVE mode
                        raw = tpool.tile([P, TCH], bf16, tag="rope_raw")
                        nc.vector.tensor_copy(raw[:], pqk[:])
                        swp = tpool.tile([P, TCH], bf16, tag="rope_swp")
                        nc.sync.dma_start(swp[0:64, :], raw[64:128, :])
                        nc.sync.dma_start(swp[64:128, :], raw[0:64, :])
                        tmpa = tpool.tile([P, TCH], bf16, tag="rope_tmpa")
                        tmpb = tpool.tile([P, TCH], bf16, tag="rope_tmpb")
                        seg = qk_rt[:, fb, t0 : t0 + TCH]
                        nc.vector.tensor_mul(tmpa[:], raw[:], cc_sb[:, t0 : t0 + TCH])
                        nc.vector.tensor_mul(tmpb[:], swp[:], ss_sb[:, t0 : t0 + TCH])
                        nc.vector.tensor_add(seg, tmpa[:], tmpb[:])
                        # RMS stats: sum of squares over head dim (partitions)
                        sq = tpool.tile([P, TCH], bf16, tag="sq")
                        nc.vector.tensor_mul(sq[:], seg, seg)
                        pms = psms.tile([1, TCH], f32, tag="ps_ms")
                        nc.tensor.matmul(pms[:], ones_col[:], sq[:], start=True, stop=True)
                        # rstd = exp(-0.5 * ln(ms/D + eps)) — both on ACT, off the PE path
                        lnms = tpool.tile([1, TCH], f32, tag="lnms")
                        nc.scalar.activation(
                            lnms[:], pms[:], AF.Ln, bias=eps_sb[0:1, :], scale=1.0 / D
                        )
                        rstd = tpool.tile([1, TCH], f32r, tag="rstd")
                        nc.scalar.activation(rstd[:], lnms[:], AF.Exp, scale=-0.5)
                        sqs.append((seg, rstd))
                    # RMS apply pass — bcast matmuls run a full block later so the
                    # ACT chain has drained and the PE never head-of-line blocks
                    for seg, rstd in sqs:
                        pb = psrb.tile([P, TCH], f32, tag="ps_b")
                        nc.tensor.matmul(pb[:], ones_row[:], rstd[:], start=True, stop=True)
                        nc.vector.tensor_mul(seg, seg, pb[:])
                    # vT blocks -> transpose -> V natural
                    for vfb in range(NK):
                        w_ap = wv_sb[:, :, vfb * D : (vfb + 1) * D]
                        pvt = ps1.tile([P, TCH], f32, tag="ps_qkv")
                        for ko in range(KO):
                            nc.tensor.matmul(
                                pvt[:], w_ap[:, ko], xt[:, ko, :],
                                start=(ko == 0), stop=(ko == KO - 1),
                            )
                        vt_sb = tpool.tile([P, TCH], bf16, tag="vt_sb")
                        nc.vector.tensor_copy(vt_sb[:], pvt[:])
                        for tb in range(TCH // P):
                            ptr = pstr.tile([P, P], bf16, tag="ps_tr")
                            nc.tensor.transpose(
                                ptr[:], vt_sb[:, tb * P : (tb + 1) * P], id_sb[:]
                            )
                            nc.vector.tensor_copy(
                                v_sb[:, tch * (TCH // P) + tb, vfb * D : (vfb + 1) * D],
                                ptr[:],
                            )

            # ---------------- Phase 3: attention + Phase 4: output projection ------------
            with (
                tc.tile_pool(name="ph3s", bufs=1) as p3s,
                tc.tile_pool(name="ph3t", bufs=3) as p3,
                tc.tile_pool(name="ph3y", bufs=3) as p3y,
            ):
                ot_sb = p3s.tile([P, NQ, T], bf16, tag="ot_sb")
                mask_sb = p3s.tile([P, 4, SPAN], bf16, tag="mask_sb")
                wo_sb = p3s.tile([P, NQ, C], bf16, tag="wo_sb")
                nc.sync.dma_start(mask_sb[:], maskT[:, :, :])
                for ko in range(NQ):
                    nc.sync.dma_start(wo_sb[:, ko, :], wo_r[:, ko, :])

                with (
                    tc.tile_pool(name="ph3sc", bufs=2, space="PSUM") as ps_sc,
                    tc.tile_pool(name="ph3ot", bufs=1, space="PSUM") as ps_ot,
                    tc.tile_pool(name="ph3nm", bufs=1, space="PSUM") as ps_nm,
                    tc.tile_pool(name="ph3yp", bufs=2, space="PSUM") as ps_yp,
                ):
                    yi = 0  # global proj-group counter (for engine alternation)

                    def emit_proj_group(tb, nch):
                        nonlocal yi
                        yps = ps_yp.tile([P, 512], f32, tag="yps")
                        for hh in range(NQ):
                            nc.tensor.matmul(
                                yps[:],
                                ot_sb[:, hh, tb * P : (tb + 1) * P],
                                wo_sb[:, hh, nch * 512 : (nch + 1) * 512],
                                start=(hh == 0), stop=(hh == NQ - 1),
                            )
                        ysb = p3y.tile([P, 512], bf16, tag="ysb")
                        # alternate the PSUM->SBUF evacuation between DVE and ACT
                        if yi % 2 == 0:
                            nc.vector.tensor_copy(ysb[:], yps[:])
                        else:
                            nc.scalar.activation(ysb[:], yps[:], AF.Copy)
                        nc.sync.dma_start(
                            y[tb * P : (tb + 1) * P, nch * 512 : (nch + 1) * 512],
                            ysb[:],
                        )
                        yi += 1

                    for s in range(NSPAN):
                        q0 = s * SPAN
                        nkb = 4 * s + 4
                        ng = nkb // 2       # score/exp groups of 2 key-blocks
                        for h in range(NQ):
                            j = h // 2
                            ot_ps = ps_ot.tile([P, SPAN], f32, tag="ot_ps")
                            sum_ps = ps_nm.tile([1, SPAN], f32, tag="nm")
                            q_ap = qk_rt[:, h, q0 : q0 + SPAN]
                            for g in range(ng):
                                sc = ps_sc.tile([P, 2, SPAN], f32, tag="sc")
                                for jj in range(2):
                                    kb = 2 * g + jj
                                    nc.tensor.matmul(
                                        sc[:, jj, :],
                                        qk_rt[:, NQ + j, kb * P : (kb + 1) * P],
                                        q_ap,
                                        start=True, stop=True,
                                    )
                                pt = p3.tile([P, 2, SPAN], bf16, tag="pt")
                                nc.scalar.activation(
                                    pt[:, :, :], sc[:, :, :], AF.Exp, scale=SCALE
                                )
                                if g >= 2 * s:  # diagonal groups: causal mask
                                    r = 2 * g - 4 * s
                                    nc.vector.tensor_mul(
                                        pt[:, :, :], pt[:, :, :],
                                        mask_sb[:, r : r + 2, :],
                                    )
                                # AV accumulation
                                for jj in range(2):
                                    kb = 2 * g + jj
                                    nc.tensor.matmul(
                                        ot_ps[:],
                                        v_sb[:, kb, j * D : (j + 1) * D],
                                        pt[:, jj, :],
                                        start=(kb == 0), stop=(kb == nkb - 1),
                                        skip_group_check=True,
                                    )
                                # row-sum: pair-add on DVE, one matmul per group
                                lf = p3.tile([P, SPAN], bf16, tag="lf")
                                nc.vector.tensor_add(lf[:], pt[:, 0, :], pt[:, 1, :])
                                nc.tensor.matmul(
                                    sum_ps[:], ones_col[:], lf[:],
                                    start=(g == 0), stop=(g == ng - 1),
                                    skip_group_check=True,
                                )
                            # normalization: 1/sums on DVE, broadcast via PE, apply
                            rec = p3.tile([1, SPAN], f32r, tag="rec")
                            with nc.allow_low_precision(
                                reason="f32r carries full fp32 payload"
                            ):
                                nc.vector.reciprocal(rec[:], sum_ps[:])
                            bc_ps = ps_nm.tile([P, SPAN], f32, tag="nm")
                            nc.tensor.matmul(
                                bc_ps[:], ones_row[:], rec[:], start=True, stop=True
                            )
                            bc_sb = p3.tile([P, SPAN], f32, tag="bc_sb")
                            nc.scalar.activation(bc_sb[:], bc_ps[:], AF.Copy)
                            nc.vector.tensor_mul(
                                ot_sb[:, h, q0 : q0 + SPAN], ot_ps[:], bc_sb[:]
                            )
                            # output projection of the previous span, interleaved
                            # as PE filler while ACT crunches the next head's exps
                            if s >= 1:
                                tb = 4 * (s - 1) + h
                                for nch in range(C // 512):
                                    emit_proj_group(tb, nch)

                    # tail: projection for the last span
                    for tb in range(4 * (NSPAN - 1), 4 * NSPAN):
                        for nch in range(C // 512):
                            emit_proj_group(tb, nch)
    nc.compile()
    return nc


_NC_CACHE = None


def _get_nc():
    global _NC_CACHE
    if _NC_CACHE is None:
        _NC_CACHE = build()
    return _NC_CACHE


def _host_inputs(x, cos, sin, wq, wk, wv, wo):
    """Build the 8 per-core input maps."""
    bft = ml_dtypes.bfloat16
    cosT = np.ascontiguousarray(cos[0, :, 0, :].T).astype(np.float32)  # (64, T)
    sinT = np.ascontiguousarray(sin[0, :, 0, :].T).astype(np.float32)
    cc = np.concatenate([cosT, cosT], axis=0).astype(bft)          # (128, T)
    ss = np.concatenate([sinT, -sinT], axis=0).astype(bft)
    # maskT[r][k, q] = 1 if q >= 128*r + k  (within a 512-q span, k-block offset r)
    qidx = np.arange(SPAN)[None, None, :]
    kidx = np.arange(P)[:, None, None]
    ridx = np.arange(4)[None, :, None]
    maskT = (qidx >= P * ridx + kidx).astype(bft)  # (128, 4, 512)
    ident = np.eye(P, dtype=np.float32).astype(bft)

    xTs = [np.ascontiguousarray(x[b].T).astype(bft) for b in range(2)]
    wq16 = wq.astype(bft)
    wk16 = wk.astype(bft)
    wv16 = wv.astype(bft)
    wo16 = wo.astype(bft)
    in_maps = []
    for c in range(8):
        b, tp = divmod(c, 4)
        in_maps.append(
            {
                "xT": xTs[b],
                "wq": np.ascontiguousarray(wq16[:, tp * FQ : (tp + 1) * FQ]),
                "wk": np.ascontiguousarray(wk16[:, tp * FK : (tp + 1) * FK]),
                "wv": np.ascontiguousarray(wv16[:, tp * FK : (tp + 1) * FK]),
                "wo": np.ascontiguousarray(wo16[tp * FQ : (tp + 1) * FQ, :]),
                "cc": cc,
                "ss": ss,
                "maskT": maskT,
                "ident": ident,
            }
        )
    return in_maps


def kernel(x, cos, sin, wq, wk, wv, wo, trace=False):
    x = np.asarray(x, dtype=np.float32)
    cos = np.asarray(cos, dtype=np.float32)
    sin = np.asarray(sin, dtype=np.float32)
    wq = np.asarray(wq, dtype=np.float32)
    wk = np.asarray(wk, dtype=np.float32)
    wv = np.asarray(wv, dtype=np.float32)
    wo = np.asarray(wo, dtype=np.float32)

    nc = _get_nc()
    in_maps = _host_inputs(x, cos, sin, wq, wk, wv, wo)
    res = run_bass_kernel_spmd(nc, in_maps, core_ids=list(range(8)), trace=trace)
    out = np.zeros((2, T, C), dtype=np.float32)
    for c in range(8):
        b = c // 4
        out[b] += res.results[c]["y"].astype(np.float32)
    if trace:
        return out, res
    return out


# revision 7
# speedup vs baseline: 1.1189x; 1.0561x over previous
"""Causal self-attention (RoPE + QK-RMSNorm, GQA 16q/8kv) Trainium2 Bass kernel.

Sharding: 8 cores = 2 batch x 4 tensor-parallel. Core c handles batch b=c//4 and
q-heads [4*tp, 4*tp+4), kv-heads [2*tp, 2*tp+2) where tp=c%4. Each core returns a
partial (T, C) output = O_heads @ wo[rows of its heads]; host sums the 4 partials
per batch (the "all-reduce after c_proj").

Matmuls run in bf16 (fp32 PSUM accumulation); softmax row-sum normalization and
RMS statistics stay in fp32/fp32r.

Phase-3 structure: scores for 2 key-blocks land in one 2-bank PSUM tile, one exp
instruction covers both; row-sums use a DVE pair-add + one accumulating matmul
per group; projection matmuls of the previous span interleave into the current
span's attention stream to keep the PE dense (HAM stays at full clock).
"""
import sys
import math

sys.path.insert(0, "/opt/trn_rl_repo")

import numpy as np
import ml_dtypes
import concourse.bacc as bacc
import concourse.mybir as mybir
import concourse.tile as tile
from concourse.bass_utils import run_bass_kernel_spmd

P = 128
T = 2048
C = 2048
KO = C // P          # 16 contraction tiles
D = 128              # head dim
NQ = 4               # q heads per core
NK = 2               # kv heads per core
NF = NQ + NK         # 6 rope/rms feature blocks (4 q + 2 k)
FQ = NQ * D          # 512
FK = NK * D          # 256
TCH = 512            # phase-1 T-chunk
NCHUNK = T // TCH    # 4
SPAN = 512           # attention q-span
NSPAN = T // SPAN    # 4
KB = T // P          # 16 key blocks
SCALE = 1.0 / math.sqrt(D)
EPS = 1.1920929e-07

f32 = mybir.dt.float32
f32r = mybir.dt.float32r
bf16 = mybir.dt.bfloat16

AF = mybir.ActivationFunctionType


def build():
    nc = bacc.Bacc("TRN2", target_bir_lowering=False)
    xT = nc.dram_tensor("xT", (C, T), bf16, kind="ExternalInput")
    wq = nc.dram_tensor("wq", (C, FQ), bf16, kind="ExternalInput")
    wk = nc.dram_tensor("wk", (C, FK), bf16, kind="ExternalInput")
    wv = nc.dram_tensor("wv", (C, FK), bf16, kind="ExternalInput")
    wo = nc.dram_tensor("wo", (FQ, C), bf16, kind="ExternalInput")
    cc = nc.dram_tensor("cc", (P, T), bf16, kind="ExternalInput")    # [cos; cos]
    ss = nc.dram_tensor("ss", (P, T), bf16, kind="ExternalInput")    # [sin; -sin]
    maskT = nc.dram_tensor("maskT", (P, 4, SPAN), bf16, kind="ExternalInput")
    ident = nc.dram_tensor("ident", (P, P), bf16, kind="ExternalInput")
    y = nc.dram_tensor("y", (T, C), bf16, kind="ExternalOutput")

    xT_r = xT.rearrange("(ko p) t -> p ko t", p=P)
    wq_r = wq.rearrange("(ko p) f -> p ko f", p=P)
    wk_r = wk.rearrange("(ko p) f -> p ko f", p=P)
    wv_r = wv.rearrange("(ko p) f -> p ko f", p=P)
    wo_r = wo.rearrange("(ko p) n -> p ko n", p=P)

    with tile.TileContext(nc) as tc:
        with tc.tile_pool(name="persist", bufs=1) as persist:
            # persistent across phases
            qk_rt = persist.tile([P, NF, T], bf16, tag="qk_rt")   # roped+normed qT/kT
            v_sb = persist.tile([P, KB, FK], bf16, tag="v_sb")    # V natural [t-part, kb, feat]
            cc_sb = persist.tile([P, T], bf16, tag="cc_sb")
            ss_sb = persist.tile([P, T], bf16, tag="ss_sb")
            id_sb = persist.tile([P, P], bf16, tag="id_sb")
            ones_col = persist.tile([P, 1], bf16, tag="ones_col")    # sums lhsT
            eps_sb = persist.tile([P, 1], f32, tag="eps_sb")
            zero_sb = persist.tile([1, 1], f32, tag="zero_sb")
            nc.vector.memset(zero_sb[:], 0.0)
            ones_f32 = persist.tile([P, 1], f32, tag="ones_f32")
            ones_row_f32 = persist.tile([1, P], f32, tag="ones_row_f32")
            nc.sync.dma_start(cc_sb[:], cc[:, :])
            nc.sync.dma_start(ss_sb[:], ss[:, :])
            nc.sync.dma_start(id_sb[:], ident[:, :])
            nc.vector.memset(eps_sb[:], EPS)
            nc.vector.memset(ones_f32[:], 1.0)
            nc.vector.memset(ones_row_f32[:], 1.0)
            nc.vector.tensor_copy(ones_col[:], ones_f32[:])

            # ------- Phase 1: QKV projections + RoPE + RMS norm + V transpose -------
            with (
                tc.tile_pool(name="ph1w", bufs=1) as wpool,
                tc.tile_pool(name="ph1x", bufs=2) as xpool,
                tc.tile_pool(name="ph1t", bufs=3) as tpool,
                tc.tile_pool(name="ph1ps", bufs=2, space="PSUM") as ps1,
                tc.tile_pool(name="ph1tr", bufs=2, space="PSUM") as pstr,
                tc.tile_pool(name="ph1ms", bufs=1, space="PSUM") as psms,
                tc.tile_pool(name="ph1rb", bufs=3, space="PSUM") as psrb,
            ):
                wq_sb = wpool.tile([P, KO, FQ], bf16, tag="wq_sb")
                wk_sb = wpool.tile([P, KO, FK], bf16, tag="wk_sb")
                wv_sb = wpool.tile([P, KO, FK], bf16, tag="wv_sb")
                # per-ko weight DMAs so the first matmuls can start early
                for ko in range(KO):
                    nc.sync.dma_start(wq_sb[:, ko, :], wq_r[:, ko, :])
                    nc.sync.dma_start(wk_sb[:, ko, :], wk_r[:, ko, :])
                    nc.sync.dma_start(wv_sb[:, ko, :], wv_r[:, ko, :])

                for tch in range(NCHUNK):
                    t0 = tch * TCH
                    xt = xpool.tile([P, KO, TCH], bf16, tag="xt")
                    # per-ko DMAs so matmuls can start as slices land
                    for ko in range(KO):
                        nc.sync.dma_start(xt[:, ko, :], xT_r[:, ko, t0 : t0 + TCH])
                    # qT / kT feature blocks (4 q heads + 2 k heads)
                    sqs = []
                    for fb in range(NF):
                        if fb < NQ:
                            w_ap = wq_sb[:, :, fb * D : (fb + 1) * D]
                        else:
                            w_ap = wk_sb[:, :, (fb - NQ) * D : (fb - NQ + 1) * D]
                        pqk = ps1.tile([P, TCH], f32, tag="ps_qkv")
                        for ko in range(KO):
                            nc.tensor.matmul(
                                pqk[:], w_ap[:, ko], xt[:, ko, :],
                                start=(ko == 0), stop=(ko == KO - 1),
                            )
                        # rope: raw chunk + half-swapped chunk (bf16), 2x DVE mode
                        raw = tpool.tile([P, TCH], bf16, tag="rope_raw")
                        nc.vector.tensor_copy(raw[:], pqk[:])
                        swp = tpool.tile([P, TCH], bf16, tag="rope_swp")
                        nc.sync.dma_start(swp[0:64, :], raw[64:128, :])
                        nc.sync.dma_start(swp[64:128, :], raw[0:64, :])
                        tmpa = tpool.tile([P, TCH], bf16, tag="rope_tmpa")
                        tmpb = tpool.tile([P, TCH], bf16, tag="rope_tmpb")
                        seg = qk_rt[:, fb, t0 : t0 + TCH]
                        nc.vector.tensor_mul(tmpa[:], raw[:], cc_sb[:, t0 : t0 + TCH])
                        nc.vector.tensor_mul(tmpb[:], swp[:], ss_sb[:, t0 : t0 + TCH])
                        nc.vector.tensor_add(seg, tmpa[:], tmpb[:])
                        # RMS stats: sum of squares over head dim (partitions)
                        sq = tpool.tile([P, TCH], bf16, tag="sq")
                        nc.vector.tensor_mul(sq[:], seg, seg)
                        pms = psms.tile([1, TCH], f32, tag="ps_ms")
                        nc.tensor.matmul(pms[:], ones_col[:], sq[:], start=True, stop=True)
                        # rstd = 1/sqrt(ms/D + eps): ACT Sqrt + fast DVE recip.
                        # (Ln+Exp here would thrash ACT table sets against the
                        # attention phase's Exp — 1.28us per switch.)
                        sqms = tpool.tile([1, TCH], f32, tag="sqms")
                        nc.scalar.activation(
                            sqms[:], pms[:], AF.Sqrt, bias=eps_sb[0:1, :], scale=1.0 / D
                        )
                        rstd = tpool.tile([1, TCH], f32, tag="rstd")
                        nc.vector.reciprocal_approx_fast(rstd[:], sqms[:])
                        sqs.append((seg, rstd))
                    # RMS apply pass — bcast matmuls run a full block later so the
                    # ACT chain has drained and the PE never head-of-line blocks
                    for seg, rstd in sqs:
                        pb = psrb.tile([P, TCH], f32, tag="ps_b")
                        nc.tensor.matmul(
                            pb[:], ones_row_f32[:], rstd[:],
                            start=True, stop=True,
                        )
                        nc.vector.tensor_mul(seg, seg, pb[:])
                    # vT blocks -> transpose -> V natural
                    for vfb in range(NK):
                        w_ap = wv_sb[:, :, vfb * D : (vfb + 1) * D]
                        pvt = ps1.tile([P, TCH], f32, tag="ps_qkv")
                        for ko in range(KO):
                            nc.tensor.matmul(
                                pvt[:], w_ap[:, ko], xt[:, ko, :],
                                start=(ko == 0), stop=(ko == KO - 1),
                            )
                        vt_sb = tpool.tile([P, TCH], bf16, tag="vt_sb")
                        nc.vector.tensor_copy(vt_sb[:], pvt[:])
                        for tb in range(TCH // P):
                            ptr = pstr.tile([P, P], bf16, tag="ps_tr")
                            nc.tensor.transpose(
                                ptr[:], vt_sb[:, tb * P : (tb + 1) * P], id_sb[:]
                            )
                            nc.vector.tensor_copy(
                                v_sb[:, tch * (TCH // P) + tb, vfb * D : (vfb + 1) * D],
                                ptr[:],
                            )

            # ---------------- Phase 3: attention + Phase 4: output projection ------------
            with (
                tc.tile_pool(name="ph3s", bufs=1) as p3s,
                tc.tile_pool(name="ph3t", bufs=3) as p3,
                tc.tile_pool(name="ph3y", bufs=3) as p3y,
            ):
                ot_sb = p3s.tile([P, NQ, T], bf16, tag="ot_sb")
                mask_sb = p3s.tile([P, 4, SPAN], bf16, tag="mask_sb")
                wo_sb = p3s.tile([P, NQ, C], bf16, tag="wo_sb")
                nc.sync.dma_start(mask_sb[:], maskT[:, :, :])
                for ko in range(NQ):
                    nc.sync.dma_start(wo_sb[:, ko, :], wo_r[:, ko, :])

                with (
                    tc.tile_pool(name="ph3sc", bufs=2, space="PSUM") as ps_sc,
                    tc.tile_pool(name="ph3ot", bufs=1, space="PSUM") as ps_ot,
                    tc.tile_pool(name="ph3nm", bufs=1, space="PSUM") as ps_nm,
                    tc.tile_pool(name="ph3yp", bufs=2, space="PSUM") as ps_yp,
                ):
                    yi = 0  # global proj-group counter (for engine alternation)

                    def emit_proj_group(tb, nch):
                        nonlocal yi
                        yps = ps_yp.tile([P, 512], f32, tag="yps")
                        for hh in range(NQ):
                            nc.tensor.matmul(
                                yps[:],
                                ot_sb[:, hh, tb * P : (tb + 1) * P],
                                wo_sb[:, hh, nch * 512 : (nch + 1) * 512],
                                start=(hh == 0), stop=(hh == NQ - 1),
                            )
                        ysb = p3y.tile([P, 512], bf16, tag="ysb")
                        # alternate the PSUM->SBUF evacuation between DVE and ACT
                        if yi % 2 == 0:
                            nc.vector.tensor_copy(ysb[:], yps[:])
                        else:
                            nc.scalar.activation(ysb[:], yps[:], AF.Copy)
                        nc.sync.dma_start(
                            y[tb * P : (tb + 1) * P, nch * 512 : (nch + 1) * 512],
                            ysb[:],
                        )
                        yi += 1

                    for s in range(NSPAN):
                        q0 = s * SPAN
                        nkb = 4 * s + 4
                        ng = nkb // 2       # score/exp groups of 2 key-blocks
                        for h in range(NQ):
                            j = h // 2
                            ot_ps = ps_ot.tile([P, SPAN], f32, tag="ot_ps")
                            sum_ps = ps_nm.tile([1, SPAN], f32, tag="nm")
                            q_ap = qk_rt[:, h, q0 : q0 + SPAN]
                            for g in range(ng):
                                sc = ps_sc.tile([P, 2, SPAN], f32, tag="sc")
                                for jj in range(2):
                                    kb = 2 * g + jj
                                    nc.tensor.matmul(
                                        sc[:, jj, :],
                                        qk_rt[:, NQ + j, kb * P : (kb + 1) * P],
                                        q_ap,
                                        start=True, stop=True,
                                    )
                                pt = p3.tile([P, 2, SPAN], bf16, tag="pt")
                                nc.scalar.activation(
                                    pt[:, :, :], sc[:, :, :], AF.Exp, scale=SCALE
                                )
                                if g >= 2 * s:  # diagonal groups: causal mask
                                    r = 2 * g - 4 * s
                                    nc.vector.tensor_mul(
                                        pt[:, :, :], pt[:, :, :],
                                        mask_sb[:, r : r + 2, :],
                                    )
                                # AV accumulation
                                for jj in range(2):
                                    kb = 2 * g + jj
                                    nc.tensor.matmul(
                                        ot_ps[:],
                                        v_sb[:, kb, j * D : (j + 1) * D],
                                        pt[:, jj, :],
                                        start=(kb == 0), stop=(kb == nkb - 1),
                                        skip_group_check=True,
                                    )
                                # row-sum: pair-add on DVE, one matmul per group
                                lf = p3.tile([P, SPAN], bf16, tag="lf")
                                nc.vector.tensor_add(lf[:], pt[:, 0, :], pt[:, 1, :])
                                nc.tensor.matmul(
                                    sum_ps[:], ones_col[:], lf[:],
                                    start=(g == 0), stop=(g == ng - 1),
                                    skip_group_check=True,
                                )
                            # normalization: fast 1/sums on DVE, broadcast via PE
                            rec = p3.tile([1, SPAN], f32, tag="rec")
                            nc.vector.reciprocal_approx_fast(rec[:], sum_ps[:])
                            bc_ps = ps_nm.tile([P, SPAN], f32, tag="nm")
                            nc.tensor.matmul(
                                bc_ps[:], ones_row_f32[:], rec[:],
                                start=True, stop=True,
                            )
                            bc_sb = p3.tile([P, SPAN], f32, tag="bc_sb")
                            nc.scalar.activation(bc_sb[:], bc_ps[:], AF.Copy)
                            nc.vector.tensor_mul(
                                ot_sb[:, h, q0 : q0 + SPAN], ot_ps[:], bc_sb[:]
                            )
                            # output projection of the previous span, interleaved
                            # as PE filler while ACT crunches the next head's exps
                            if s >= 1:
                                tb = 4 * (s - 1) + h
                                for nch in range(C // 512):
                                    emit_proj_group(tb, nch)

                    # tail: projection for the last span
                    for tb in range(4 * (NSPAN - 1), 4 * NSPAN):
                        for nch in range(C // 512):
                            emit_proj_group(tb, nch)
    nc.compile()
    return nc


_NC_CACHE = None


def _get_nc():
    global _NC_CACHE
    if _NC_CACHE is None:
        _NC_CACHE = build()
    return _NC_CACHE


def _host_inputs(x, cos, sin, wq, wk, wv, wo):
    """Build the 8 per-core input maps."""
    bft = ml_dtypes.bfloat16
    cosT = np.ascontiguousarray(cos[0, :, 0, :].T).astype(np.float32)  # (64, T)
    sinT = np.ascontiguousarray(sin[0, :, 0, :].T).astype(np.float32)
    cc = np.concatenate([cosT, cosT], axis=0).astype(bft)          # (128, T)
    ss = np.concatenate([sinT, -sinT], axis=0).astype(bft)
    # maskT[r][k, q] = 1 if q >= 128*r + k  (within a 512-q span, k-block offset r)
    qidx = np.arange(SPAN)[None, None, :]
    kidx = np.arange(P)[:, None, None]
    ridx = np.arange(4)[None, :, None]
    maskT = (qidx >= P * ridx + kidx).astype(bft)  # (128, 4, 512)
    ident = np.eye(P, dtype=np.float32).astype(bft)

    xTs = [np.ascontiguousarray(x[b].T).astype(bft) for b in range(2)]
    wq16 = wq.astype(bft)
    wk16 = wk.astype(bft)
    wv16 = wv.astype(bft)
    wo16 = wo.astype(bft)
    in_maps = []
    for c in range(8):
        b, tp = divmod(c, 4)
        in_maps.append(
            {
                "xT": xTs[b],
                "wq": np.ascontiguousarray(wq16[:, tp * FQ : (tp + 1) * FQ]),
                "wk": np.ascontiguousarray(wk16[:, tp * FK : (tp + 1) * FK]),
                "wv": np.ascontiguousarray(wv16[:, tp * FK : (tp + 1) * FK]),
                "wo": np.ascontiguousarray(wo16[tp * FQ : (tp + 1) * FQ, :]),
                "cc": cc,
                "ss": ss,
                "maskT": maskT,
                "ident": ident,
            }
        )
    return in_maps


def kernel(x, cos, sin, wq, wk, wv, wo, trace=False):
    x = np.asarray(x, dtype=np.float32)
    cos = np.asarray(cos, dtype=np.float32)
    sin = np.asarray(sin, dtype=np.float32)
    wq = np.asarray(wq, dtype=np.float32)
    wk = np.asarray(wk, dtype=np.float32)
    wv = np.asarray(wv, dtype=np.float32)
    wo = np.asarray(wo, dtype=np.float32)

    nc = _get_nc()
    in_maps = _host_inputs(x, cos, sin, wq, wk, wv, wo)
    res = run_bass_kernel_spmd(nc, in_maps, core_ids=list(range(8)), trace=trace)
    out = np.zeros((2, T, C), dtype=np.float32)
    for c in range(8):
        b = c // 4
        out[b] += res.results[c]["y"].astype(np.float32)
    if trace:
        return out, res
    return out


# revision 8
# speedup vs baseline: 1.1539x; 1.0313x over previous
"""Causal self-attention (RoPE + QK-RMSNorm, GQA 16q/8kv) Trainium2 Bass kernel.

Sharding: 8 cores = 2 batch x 4 tensor-parallel. Core c handles batch b=c//4 and
q-heads [4*tp, 4*tp+4), kv-heads [2*tp, 2*tp+2) where tp=c%4. Each core returns a
partial (T, C) output = O_heads @ wo[rows of its heads]; host sums the 4 partials
per batch (the "all-reduce after c_proj").

Matmuls run in bf16 (fp32 PSUM accumulation); softmax row-sum normalization and
RMS statistics stay in fp32/fp32r.

Phase-3 structure: scores for 2 key-blocks land in one 2-bank PSUM tile, one exp
instruction covers both; row-sums use a DVE pair-add + one accumulating matmul
per group; projection matmuls of the previous span interleave into the current
span's attention stream to keep the PE dense (HAM stays at full clock).
"""
import sys
import math

sys.path.insert(0, "/opt/trn_rl_repo")

import numpy as np
import ml_dtypes
import concourse.bacc as bacc
import concourse.mybir as mybir
import concourse.tile as tile
from concourse.bass_utils import run_bass_kernel_spmd

P = 128
T = 2048
C = 2048
KO = C // P          # 16 contraction tiles
D = 128              # head dim
NQ = 4               # q heads per core
NK = 2               # kv heads per core
NF = NQ + NK         # 6 rope/rms feature blocks (4 q + 2 k)
FQ = NQ * D          # 512
FK = NK * D          # 256
TCH = 512            # phase-1 T-chunk
NCHUNK = T // TCH    # 4
SPAN = 512           # attention q-span
NSPAN = T // SPAN    # 4
KB = T // P          # 16 key blocks
SCALE = 1.0 / math.sqrt(D)
EPS = 1.1920929e-07

f32 = mybir.dt.float32
f32r = mybir.dt.float32r
bf16 = mybir.dt.bfloat16

AF = mybir.ActivationFunctionType


def build():
    nc = bacc.Bacc("TRN2", target_bir_lowering=False)
    xT = nc.dram_tensor("xT", (C, T), bf16, kind="ExternalInput")
    wq = nc.dram_tensor("wq", (C, FQ), bf16, kind="ExternalInput")
    wk = nc.dram_tensor("wk", (C, FK), bf16, kind="ExternalInput")
    wv = nc.dram_tensor("wv", (C, FK), bf16, kind="ExternalInput")
    wo = nc.dram_tensor("wo", (FQ, C), bf16, kind="ExternalInput")
    cc = nc.dram_tensor("cc", (P, T), bf16, kind="ExternalInput")    # [cos; cos]
    ss = nc.dram_tensor("ss", (P, T), bf16, kind="ExternalInput")    # [sin; -sin]
    maskT = nc.dram_tensor("maskT", (P, 4, SPAN), bf16, kind="ExternalInput")
    ident = nc.dram_tensor("ident", (P, P), bf16, kind="ExternalInput")
    y = nc.dram_tensor("y", (T, C), bf16, kind="ExternalOutput")

    xT_r = xT.rearrange("(ko p) t -> p ko t", p=P)
    wq_r = wq.rearrange("(ko p) f -> p ko f", p=P)
    wk_r = wk.rearrange("(ko p) f -> p ko f", p=P)
    wv_r = wv.rearrange("(ko p) f -> p ko f", p=P)
    wo_r = wo.rearrange("(ko p) n -> p ko n", p=P)

    with tile.TileContext(nc) as tc:
        with tc.tile_pool(name="persist", bufs=1) as persist:
            # persistent across phases
            qk_rt = persist.tile([P, NF, T], bf16, tag="qk_rt")   # roped+normed qT/kT
            v_sb = persist.tile([P, KB, FK], bf16, tag="v_sb")    # V natural [t-part, kb, feat]
            cc_sb = persist.tile([P, T], bf16, tag="cc_sb")
            ss_sb = persist.tile([P, T], bf16, tag="ss_sb")
            id_sb = persist.tile([P, P], bf16, tag="id_sb")
            ones_col = persist.tile([P, 1], bf16, tag="ones_col")    # sums lhsT
            eps_sb = persist.tile([P, 1], f32, tag="eps_sb")
            zero_sb = persist.tile([1, 1], f32, tag="zero_sb")
            nc.vector.memset(zero_sb[:], 0.0)
            ones_f32 = persist.tile([P, 1], f32, tag="ones_f32")
            ones_row_f32 = persist.tile([1, P], f32, tag="ones_row_f32")
            nc.sync.dma_start(cc_sb[:], cc[:, :])
            nc.sync.dma_start(ss_sb[:], ss[:, :])
            nc.sync.dma_start(id_sb[:], ident[:, :])
            nc.vector.memset(eps_sb[:], EPS)
            nc.vector.memset(ones_f32[:], 1.0)
            nc.vector.memset(ones_row_f32[:], 1.0)
            nc.vector.tensor_copy(ones_col[:], ones_f32[:])

            # ------- Phase 1: QKV projections + RoPE + RMS norm + V transpose -------
            with (
                tc.tile_pool(name="ph1w", bufs=1) as wpool,
                tc.tile_pool(name="ph1x", bufs=2) as xpool,
                tc.tile_pool(name="ph1t", bufs=3) as tpool,
                tc.tile_pool(name="ph1ps", bufs=2, space="PSUM") as ps1,
                tc.tile_pool(name="ph1tr", bufs=2, space="PSUM") as pstr,
                tc.tile_pool(name="ph1ms", bufs=1, space="PSUM") as psms,
                tc.tile_pool(name="ph1rb", bufs=3, space="PSUM") as psrb,
            ):
                wq_sb = wpool.tile([P, KO, FQ], bf16, tag="wq_sb")
                wk_sb = wpool.tile([P, KO, FK], bf16, tag="wk_sb")
                wv_sb = wpool.tile([P, KO, FK], bf16, tag="wv_sb")
                # per-ko weight DMAs so the first matmuls can start early
                for ko in range(KO):
                    nc.sync.dma_start(wq_sb[:, ko, :], wq_r[:, ko, :])
                    nc.sync.dma_start(wk_sb[:, ko, :], wk_r[:, ko, :])
                    nc.sync.dma_start(wv_sb[:, ko, :], wv_r[:, ko, :])

                for tch in range(NCHUNK):
                    t0 = tch * TCH
                    xt = xpool.tile([P, KO, TCH], bf16, tag="xt")
                    # per-ko DMAs so matmuls can start as slices land
                    for ko in range(KO):
                        nc.sync.dma_start(xt[:, ko, :], xT_r[:, ko, t0 : t0 + TCH])
                    # --- pass A: all projection matmuls; evacuate PSUM fast and
                    # issue the rope swap DMAs with a full block of lead time so
                    # the in-order DVE queue never head-of-line blocks on them.
                    qk_pend = []
                    for fb in range(NF):
                        if fb < NQ:
                            w_ap = wq_sb[:, :, fb * D : (fb + 1) * D]
                        else:
                            w_ap = wk_sb[:, :, (fb - NQ) * D : (fb - NQ + 1) * D]
                        pqk = ps1.tile([P, TCH], f32, tag="ps_qkv")
                        for ko in range(KO):
                            nc.tensor.matmul(
                                pqk[:], w_ap[:, ko], xt[:, ko, :],
                                start=(ko == 0), stop=(ko == KO - 1),
                            )
                        raw = tpool.tile([P, TCH], bf16, tag="rope_raw", bufs=7)
                        nc.vector.tensor_copy(raw[:], pqk[:])
                        swp = tpool.tile([P, TCH], bf16, tag="rope_swp", bufs=7)
                        nc.sync.dma_start(swp[0:64, :], raw[64:128, :])
                        nc.sync.dma_start(swp[64:128, :], raw[0:64, :])
                        qk_pend.append((fb, raw, swp))
                    vt_pend = []
                    for vfb in range(NK):
                        w_ap = wv_sb[:, :, vfb * D : (vfb + 1) * D]
                        pvt = ps1.tile([P, TCH], f32, tag="ps_qkv")
                        for ko in range(KO):
                            nc.tensor.matmul(
                                pvt[:], w_ap[:, ko], xt[:, ko, :],
                                start=(ko == 0), stop=(ko == KO - 1),
                            )
                        vt_sb = tpool.tile([P, TCH], bf16, tag="vt_sb", bufs=3)
                        nc.vector.tensor_copy(vt_sb[:], pvt[:])
                        vt_pend.append((vfb, vt_sb))
                    # --- pass B: rope math + RMS stats (swaps have landed)
                    sqs = []
                    for fb, raw, swp in qk_pend:
                        tmpa = tpool.tile([P, TCH], bf16, tag="rope_tmpa")
                        tmpb = tpool.tile([P, TCH], bf16, tag="rope_tmpb")
                        seg = qk_rt[:, fb, t0 : t0 + TCH]
                        nc.vector.tensor_mul(tmpa[:], raw[:], cc_sb[:, t0 : t0 + TCH])
                        nc.vector.tensor_mul(tmpb[:], swp[:], ss_sb[:, t0 : t0 + TCH])
                        nc.vector.tensor_add(seg, tmpa[:], tmpb[:])
                        # RMS stats: sum of squares over head dim (partitions)
                        sq = tpool.tile([P, TCH], bf16, tag="sq")
                        nc.vector.tensor_mul(sq[:], seg, seg)
                        pms = psms.tile([1, TCH], f32, tag="ps_ms")
                        nc.tensor.matmul(pms[:], ones_col[:], sq[:], start=True, stop=True)
                        # rstd = 1/sqrt(ms/D + eps): ACT Sqrt + fast DVE recip.
                        # (Ln+Exp here would thrash ACT table sets against the
                        # attention phase's Exp — 1.28us per switch.)
                        sqms = tpool.tile([1, TCH], f32, tag="sqms", bufs=7)
                        nc.scalar.activation(
                            sqms[:], pms[:], AF.Sqrt, bias=eps_sb[0:1, :], scale=1.0 / D
                        )
                        rstd = tpool.tile([1, TCH], f32, tag="rstd", bufs=7)
                        nc.vector.reciprocal_approx_fast(rstd[:], sqms[:])
                        sqs.append((seg, rstd))
                    # --- pass C: V transposes + RMS apply
                    for vfb, vt_sb in vt_pend:
                        for tb in range(TCH // P):
                            ptr = pstr.tile([P, P], bf16, tag="ps_tr")
                            nc.tensor.transpose(
                                ptr[:], vt_sb[:, tb * P : (tb + 1) * P], id_sb[:]
                            )
                            nc.vector.tensor_copy(
                                v_sb[:, tch * (TCH // P) + tb, vfb * D : (vfb + 1) * D],
                                ptr[:],
                            )
                    for seg, rstd in sqs:
                        pb = psrb.tile([P, TCH], f32, tag="ps_b")
                        nc.tensor.matmul(
                            pb[:], ones_row_f32[:], rstd[:],
                            start=True, stop=True,
                        )
                        nc.vector.tensor_mul(seg, seg, pb[:])

            # ---------------- Phase 3: attention + Phase 4: output projection ------------
            with (
                tc.tile_pool(name="ph3s", bufs=1) as p3s,
                tc.tile_pool(name="ph3t", bufs=3) as p3,
                tc.tile_pool(name="ph3y", bufs=3) as p3y,
            ):
                ot_sb = p3s.tile([P, NQ, T], bf16, tag="ot_sb")
                mask_sb = p3s.tile([P, 4, SPAN], bf16, tag="mask_sb")
                wo_sb = p3s.tile([P, NQ, C], bf16, tag="wo_sb")
                nc.sync.dma_start(mask_sb[:], maskT[:, :, :])
                for ko in range(NQ):
                    nc.sync.dma_start(wo_sb[:, ko, :], wo_r[:, ko, :])

                with (
                    tc.tile_pool(name="ph3sc", bufs=2, space="PSUM") as ps_sc,
                    tc.tile_pool(name="ph3ot", bufs=1, space="PSUM") as ps_ot,
                    tc.tile_pool(name="ph3nm", bufs=1, space="PSUM") as ps_nm,
                    tc.tile_pool(name="ph3yp", bufs=2, space="PSUM") as ps_yp,
                ):
                    yi = 0  # global proj-group counter (for engine alternation)

                    def emit_proj_group(tb, nch):
                        nonlocal yi
                        yps = ps_yp.tile([P, 512], f32, tag="yps")
                        for hh in range(NQ):
                            nc.tensor.matmul(
                                yps[:],
                                ot_sb[:, hh, tb * P : (tb + 1) * P],
                                wo_sb[:, hh, nch * 512 : (nch + 1) * 512],
                                start=(hh == 0), stop=(hh == NQ - 1),
                            )
                        ysb = p3y.tile([P, 512], bf16, tag="ysb")
                        # alternate the PSUM->SBUF evacuation between DVE and ACT
                        if yi % 2 == 0:
                            nc.vector.tensor_copy(ysb[:], yps[:])
                        else:
                            nc.scalar.activation(ysb[:], yps[:], AF.Copy)
                        nc.sync.dma_start(
                            y[tb * P : (tb + 1) * P, nch * 512 : (nch + 1) * 512],
                            ysb[:],
                        )
                        yi += 1

                    for s in range(NSPAN):
                        q0 = s * SPAN
                        nkb = 4 * s + 4
                        ng = nkb // 2       # score/exp groups of 2 key-blocks
                        for h in range(NQ):
                            j = h // 2
                            ot_ps = ps_ot.tile([P, SPAN], f32, tag="ot_ps")
                            sum_ps = ps_nm.tile([1, SPAN], f32, tag="nm")
                            q_ap = qk_rt[:, h, q0 : q0 + SPAN]
                            for g in range(ng):
                                sc = ps_sc.tile([P, 2, SPAN], f32, tag="sc")
                                for jj in range(2):
                                    kb = 2 * g + jj
                                    nc.tensor.matmul(
                                        sc[:, jj, :],
                                        qk_rt[:, NQ + j, kb * P : (kb + 1) * P],
                                        q_ap,
                                        start=True, stop=True,
                                    )
                                pt = p3.tile([P, 2, SPAN], bf16, tag="pt")
                                nc.scalar.activation(
                                    pt[:, :, :], sc[:, :, :], AF.Exp, scale=SCALE
                                )
                                if g >= 2 * s:  # diagonal groups: causal mask
                                    r = 2 * g - 4 * s
                                    nc.vector.tensor_mul(
                                        pt[:, :, :], pt[:, :, :],
                                        mask_sb[:, r : r + 2, :],
                                    )
                                # AV accumulation
                                for jj in range(2):
                                    kb = 2 * g + jj
                                    nc.tensor.matmul(
                                        ot_ps[:],
                                        v_sb[:, kb, j * D : (j + 1) * D],
                                        pt[:, jj, :],
                                        start=(kb == 0), stop=(kb == nkb - 1),
                                        skip_group_check=True,
                                    )
                                # row-sum: pair-add on DVE, one matmul per group
                                lf = p3.tile([P, SPAN], bf16, tag="lf")
                                nc.vector.tensor_add(lf[:], pt[:, 0, :], pt[:, 1, :])
                                nc.tensor.matmul(
                                    sum_ps[:], ones_col[:], lf[:],
                                    start=(g == 0), stop=(g == ng - 1),
                                    skip_group_check=True,
                                )
                            # normalization: fast 1/sums on DVE, broadcast via PE
                            rec = p3.tile([1, SPAN], f32, tag="rec")
                            nc.vector.reciprocal_approx_fast(rec[:], sum_ps[:])
                            bc_ps = ps_nm.tile([P, SPAN], f32, tag="nm")
                            nc.tensor.matmul(
                                bc_ps[:], ones_row_f32[:], rec[:],
                                start=True, stop=True,
                            )
                            bc_sb = p3.tile([P, SPAN], f32, tag="bc_sb")
                            nc.scalar.activation(bc_sb[:], bc_ps[:], AF.Copy)
                            nc.vector.tensor_mul(
                                ot_sb[:, h, q0 : q0 + SPAN], ot_ps[:], bc_sb[:]
                            )
                            # output projection of the previous span, interleaved
                            # as PE filler while ACT crunches the next head's exps
                            if s >= 1:
                                tb = 4 * (s - 1) + h
                                for nch in range(C // 512):
                                    emit_proj_group(tb, nch)

                    # tail: projection for the last span
                    for tb in range(4 * (NSPAN - 1), 4 * NSPAN):
                        for nch in range(C // 512):
                            emit_proj_group(tb, nch)
    nc.compile()
    return nc


_NC_CACHE = None


def _get_nc():
    global _NC_CACHE
    if _NC_CACHE is None:
        _NC_CACHE = build()
    return _NC_CACHE


def _host_inputs(x, cos, sin, wq, wk, wv, wo):
    """Build the 8 per-core input maps."""
    bft = ml_dtypes.bfloat16
    cosT = np.ascontiguousarray(cos[0, :, 0, :].T).astype(np.float32)  # (64, T)
    sinT = np.ascontiguousarray(sin[0, :, 0, :].T).astype(np.float32)
    cc = np.concatenate([cosT, cosT], axis=0).astype(bft)          # (128, T)
    ss = np.concatenate([sinT, -sinT], axis=0).astype(bft)
    # maskT[r][k, q] = 1 if q >= 128*r + k  (within a 512-q span, k-block offset r)
    qidx = np.arange(SPAN)[None, None, :]
    kidx = np.arange(P)[:, None, None]
    ridx = np.arange(4)[None, :, None]
    maskT = (qidx >= P * ridx + kidx).astype(bft)  # (128, 4, 512)
    ident = np.eye(P, dtype=np.float32).astype(bft)

    xTs = [np.ascontiguousarray(x[b].T).astype(bft) for b in range(2)]
    wq16 = wq.astype(bft)
    wk16 = wk.astype(bft)
    wv16 = wv.astype(bft)
    wo16 = wo.astype(bft)
    in_maps = []
    for c in range(8):
        b, tp = divmod(c, 4)
        in_maps.append(
            {
                "xT": xTs[b],
                "wq": np.ascontiguousarray(wq16[:, tp * FQ : (tp + 1) * FQ]),
                "wk": np.ascontiguousarray(wk16[:, tp * FK : (tp + 1) * FK]),
                "wv": np.ascontiguousarray(wv16[:, tp * FK : (tp + 1) * FK]),
                "wo": np.ascontiguousarray(wo16[tp * FQ : (tp + 1) * FQ, :]),
                "cc": cc,
                "ss": ss,
                "maskT": maskT,
                "ident": ident,
            }
        )
    return in_maps


def kernel(x, cos, sin, wq, wk, wv, wo, trace=False):
    x = np.asarray(x, dtype=np.float32)
    cos = np.asarray(cos, dtype=np.float32)
    sin = np.asarray(sin, dtype=np.float32)
    wq = np.asarray(wq, dtype=np.float32)
    wk = np.asarray(wk, dtype=np.float32)
    wv = np.asarray(wv, dtype=np.float32)
    wo = np.asarray(wo, dtype=np.float32)

    nc = _get_nc()
    in_maps = _host_inputs(x, cos, sin, wq, wk, wv, wo)
    res = run_bass_kernel_spmd(nc, in_maps, core_ids=list(range(8)), trace=trace)
    out = np.zeros((2, T, C), dtype=np.float32)
    for c in range(8):
        b = c // 4
        out[b] += res.results[c]["y"].astype(np.float32)
    if trace:
        return out, res
    return out


# revision 11
# speedup vs baseline: 1.1679x; 1.0122x over previous
"""Causal self-attention (RoPE + QK-RMSNorm, GQA 16q/8kv) Trainium2 Bass kernel.

Sharding: 8 cores = 2 batch x 4 tensor-parallel. Core c handles batch b=c//4 and
q-heads [4*tp, 4*tp+4), kv-heads [2*tp, 2*tp+2) where tp=c%4. Each core returns a
partial (T, C) output = O_heads @ wo[rows of its heads]; host sums the 4 partials
per batch (the "all-reduce after c_proj").

Matmuls run in bf16 (fp32 PSUM accumulation); softmax row-sum normalization and
RMS statistics stay in fp32/fp32r.

Phase-3 structure: scores for 2 key-blocks land in one 2-bank PSUM tile, one exp
instruction covers both; row-sums use a DVE pair-add + one accumulating matmul
per group; projection matmuls of the previous span interleave into the current
span's attention stream to keep the PE dense (HAM stays at full clock).
"""
import sys
import math

sys.path.insert(0, "/opt/trn_rl_repo")

import numpy as np
import ml_dtypes
import concourse.bacc as bacc
import concourse.mybir as mybir
import concourse.tile as tile
from concourse.bass_utils import run_bass_kernel_spmd

P = 128
T = 2048
C = 2048
KO = C // P          # 16 contraction tiles
D = 128              # head dim
NQ = 4               # q heads per core
NK = 2               # kv heads per core
NF = NQ + NK         # 6 rope/rms feature blocks (4 q + 2 k)
FQ = NQ * D          # 512
FK = NK * D          # 256
TCH = 512            # phase-1 T-chunk
NCHUNK = T // TCH    # 4
SPAN = 512           # attention q-span
NSPAN = T // SPAN    # 4
KB = T // P          # 16 key blocks
SCALE = 1.0 / math.sqrt(D)
EPS = 1.1920929e-07

f32 = mybir.dt.float32
f32r = mybir.dt.float32r
bf16 = mybir.dt.bfloat16

AF = mybir.ActivationFunctionType


def build():
    nc = bacc.Bacc("TRN2", target_bir_lowering=False)
    xT = nc.dram_tensor("xT", (C, T), bf16, kind="ExternalInput")
    wq = nc.dram_tensor("wq", (C, FQ), bf16, kind="ExternalInput")
    wk = nc.dram_tensor("wk", (C, FK), bf16, kind="ExternalInput")
    wv = nc.dram_tensor("wv", (C, FK), bf16, kind="ExternalInput")
    wo = nc.dram_tensor("wo", (FQ, C), bf16, kind="ExternalInput")
    cc = nc.dram_tensor("cc", (P, T), bf16, kind="ExternalInput")    # [cos; cos]
    ss = nc.dram_tensor("ss", (P, T), bf16, kind="ExternalInput")    # [sin; -sin]
    maskT = nc.dram_tensor("maskT", (P, 4, SPAN), bf16, kind="ExternalInput")
    ident = nc.dram_tensor("ident", (P, P), bf16, kind="ExternalInput")
    y = nc.dram_tensor("y", (T, C), bf16, kind="ExternalOutput")

    xT_r = xT.rearrange("(ko p) t -> p ko t", p=P)
    wq_r = wq.rearrange("(ko p) f -> p ko f", p=P)
    wk_r = wk.rearrange("(ko p) f -> p ko f", p=P)
    wv_r = wv.rearrange("(ko p) f -> p ko f", p=P)
    wo_r = wo.rearrange("(ko p) n -> p ko n", p=P)

    with tile.TileContext(nc) as tc:
        with tc.tile_pool(name="persist", bufs=1) as persist:
            # persistent across phases
            qk_rt = persist.tile([P, NF, T], bf16, tag="qk_rt")   # roped+normed qT/kT
            v_sb = persist.tile([P, KB, FK], bf16, tag="v_sb")    # V natural [t-part, kb, feat]
            cc_sb = persist.tile([P, T], bf16, tag="cc_sb")
            ss_sb = persist.tile([P, T], bf16, tag="ss_sb")
            id_sb = persist.tile([P, P], bf16, tag="id_sb")
            ones_col = persist.tile([P, 1], bf16, tag="ones_col")    # sums lhsT
            eps_sb = persist.tile([P, 1], f32, tag="eps_sb")
            zero_sb = persist.tile([1, 1], f32, tag="zero_sb")
            nc.vector.memset(zero_sb[:], 0.0)
            ones_f32 = persist.tile([P, 1], f32, tag="ones_f32")
            ones_row_f32 = persist.tile([1, P], f32, tag="ones_row_f32")
            nc.sync.dma_start(cc_sb[:], cc[:, :])
            nc.sync.dma_start(ss_sb[:], ss[:, :])
            nc.sync.dma_start(id_sb[:], ident[:, :])
            nc.vector.memset(eps_sb[:], EPS)
            nc.vector.memset(ones_f32[:], 1.0)
            nc.vector.memset(ones_row_f32[:], 1.0)
            nc.vector.tensor_copy(ones_col[:], ones_f32[:])

            # ------- Phase 1: QKV projections + RoPE + RMS norm + V transpose -------
            with (
                tc.tile_pool(name="ph1w", bufs=1) as wpool,
                tc.tile_pool(name="ph1x", bufs=2) as xpool,
                tc.tile_pool(name="ph1t", bufs=3) as tpool,
                tc.tile_pool(name="ph1ps", bufs=3, space="PSUM") as ps1,
                tc.tile_pool(name="ph1tr", bufs=2, space="PSUM") as pstr,
                tc.tile_pool(name="ph1ms", bufs=1, space="PSUM") as psms,
                tc.tile_pool(name="ph1rb", bufs=2, space="PSUM") as psrb,
            ):
                wq_sb = wpool.tile([P, KO, FQ], bf16, tag="wq_sb")
                wk_sb = wpool.tile([P, KO, FK], bf16, tag="wk_sb")
                wv_sb = wpool.tile([P, KO, FK], bf16, tag="wv_sb")
                # per-ko weight DMAs so the first matmuls can start early
                for ko in range(KO):
                    nc.sync.dma_start(wq_sb[:, ko, :], wq_r[:, ko, :])
                    nc.sync.dma_start(wk_sb[:, ko, :], wk_r[:, ko, :])
                    nc.sync.dma_start(wv_sb[:, ko, :], wv_r[:, ko, :])

                # Software-pipelined over feature blocks, carried across chunk
                # boundaries: each block's rope/RMS/transpose post-processing is
                # deferred by 2 matmul groups (~7us of PE cover) so the in-order
                # DVE queue and the swap DMAs never stall the PE. The RMS
                # broadcast+apply trails one further post-step.
                posts = []   # deferred post-process closures
                appls = []   # deferred RMS broadcast+apply

                def emit_post_qk(args):
                    fb, tch, raw, swp = args
                    t0 = tch * TCH
                    tmpa = tpool.tile([P, TCH], bf16, tag="rope_tmpa")
                    tmpb = tpool.tile([P, TCH], bf16, tag="rope_tmpb")
                    seg = qk_rt[:, fb, t0 : t0 + TCH]
                    nc.vector.tensor_mul(tmpa[:], raw[:], cc_sb[:, t0 : t0 + TCH])
                    nc.vector.tensor_mul(tmpb[:], swp[:], ss_sb[:, t0 : t0 + TCH])
                    nc.vector.tensor_add(seg, tmpa[:], tmpb[:])
                    # RMS stats: sum of squares over head dim (partitions)
                    sq = tpool.tile([P, TCH], bf16, tag="sq")
                    nc.vector.tensor_mul(sq[:], seg, seg)
                    pms = psms.tile([1, TCH], f32, tag="ps_ms")
                    nc.tensor.matmul(pms[:], ones_col[:], sq[:], start=True, stop=True)
                    # rstd = 1/sqrt(ms/D + eps): ACT Sqrt + fast DVE recip.
                    # (Ln+Exp here would thrash ACT table sets against the
                    # attention phase's Exp — 1.28us per switch.)
                    sqms = tpool.tile([1, TCH], f32, tag="sqms", bufs=5)
                    nc.scalar.activation(
                        sqms[:], pms[:], AF.Sqrt, bias=eps_sb[0:1, :], scale=1.0 / D
                    )
                    rstd = tpool.tile([1, TCH], f32, tag="rstd", bufs=5)
                    nc.vector.reciprocal_approx_fast(rstd[:], sqms[:])
                    appls.append((seg, rstd))

                def emit_post_v(args):
                    vfb, tch, vt_sb = args
                    ptr = pstr.tile([P, TCH // P, P], bf16, tag="ps_tr")
                    for tb in range(TCH // P):
                        nc.tensor.transpose(
                            ptr[:, tb, :],
                            vt_sb[:, tb * P : (tb + 1) * P],
                            id_sb[:],
                        )
                    nc.vector.tensor_copy(
                        v_sb[
                            :,
                            tch * (TCH // P) : (tch + 1) * (TCH // P),
                            vfb * D : (vfb + 1) * D,
                        ],
                        ptr[:, :, :],
                    )

                def drain_one_post():
                    kind, args = posts.pop(0)
                    (emit_post_qk if kind == "qk" else emit_post_v)(args)
                    if len(appls) > 1:
                        seg, rstd = appls.pop(0)
                        pb = psrb.tile([P, TCH], f32, tag="ps_b")
                        nc.tensor.matmul(
                            pb[:], ones_row_f32[:], rstd[:], start=True, stop=True
                        )
                        nc.vector.tensor_mul(seg, seg, pb[:])

                for tch in range(NCHUNK):
                    t0 = tch * TCH
                    xt = xpool.tile([P, KO, TCH], bf16, tag="xt", bufs=3)
                    # per-ko DMAs so matmuls can start as slices land
                    for ko in range(KO):
                        nc.sync.dma_start(xt[:, ko, :], xT_r[:, ko, t0 : t0 + TCH])
                    # v first: its post (PE transposes) is cheap and swap-free
                    for vfb in range(NK):
                        w_ap = wv_sb[:, :, vfb * D : (vfb + 1) * D]
                        pvt = ps1.tile([P, TCH], f32, tag="ps_qkv")
                        for ko in range(KO):
                            nc.tensor.matmul(
                                pvt[:], w_ap[:, ko], xt[:, ko, :],
                                start=(ko == 0), stop=(ko == KO - 1),
                            )
                        vt_sb = tpool.tile([P, TCH], bf16, tag="vt_sb", bufs=3)
                        nc.vector.tensor_copy(vt_sb[:], pvt[:])
                        posts.append(("v", (vfb, tch, vt_sb)))
                        if len(posts) > 2:
                            drain_one_post()
                    for fb in range(NF):
                        if fb < NQ:
                            w_ap = wq_sb[:, :, fb * D : (fb + 1) * D]
                        else:
                            w_ap = wk_sb[:, :, (fb - NQ) * D : (fb - NQ + 1) * D]
                        pqk = ps1.tile([P, TCH], f32, tag="ps_qkv")
                        for ko in range(KO):
                            nc.tensor.matmul(
                                pqk[:], w_ap[:, ko], xt[:, ko, :],
                                start=(ko == 0), stop=(ko == KO - 1),
                            )
                        raw = tpool.tile([P, TCH], bf16, tag="rope_raw", bufs=5)
                        nc.vector.tensor_copy(raw[:], pqk[:])
                        swp = tpool.tile([P, TCH], bf16, tag="rope_swp", bufs=5)
                        nc.sync.dma_start(swp[0:64, :], raw[64:128, :])
                        nc.sync.dma_start(swp[64:128, :], raw[0:64, :])
                        posts.append(("qk", (fb, tch, raw, swp)))
                        if len(posts) > 2:
                            drain_one_post()
                while posts:
                    drain_one_post()
                while appls:
                    seg, rstd = appls.pop(0)
                    pb = psrb.tile([P, TCH], f32, tag="ps_b")
                    nc.tensor.matmul(
                        pb[:], ones_row_f32[:], rstd[:], start=True, stop=True
                    )
                    nc.vector.tensor_mul(seg, seg, pb[:])

            # ---------------- Phase 3: attention + Phase 4: output projection ------------
            with (
                tc.tile_pool(name="ph3s", bufs=1) as p3s,
                tc.tile_pool(name="ph3t", bufs=3) as p3,
                tc.tile_pool(name="ph3y", bufs=3) as p3y,
            ):
                ot_sb = p3s.tile([P, NQ, T], bf16, tag="ot_sb")
                mask_sb = p3s.tile([P, 4, SPAN], bf16, tag="mask_sb")
                wo_sb = p3s.tile([P, NQ, C], bf16, tag="wo_sb")
                nc.sync.dma_start(mask_sb[:], maskT[:, :, :])
                for ko in range(NQ):
                    nc.sync.dma_start(wo_sb[:, ko, :], wo_r[:, ko, :])

                with (
                    tc.tile_pool(name="ph3sc", bufs=2, space="PSUM") as ps_sc,
                    tc.tile_pool(name="ph3ot", bufs=1, space="PSUM") as ps_ot,
                    tc.tile_pool(name="ph3nm", bufs=1, space="PSUM") as ps_nm,
                    tc.tile_pool(name="ph3yp", bufs=2, space="PSUM") as ps_yp,
                ):
                    yi = 0  # global proj-group counter (for engine alternation)

                    def emit_proj_group(tb, nch):
                        nonlocal yi
                        yps = ps_yp.tile([P, 512], f32, tag="yps")
                        for hh in range(NQ):
                            nc.tensor.matmul(
                                yps[:],
                                ot_sb[:, hh, tb * P : (tb + 1) * P],
                                wo_sb[:, hh, nch * 512 : (nch + 1) * 512],
                                start=(hh == 0), stop=(hh == NQ - 1),
                            )
                        ysb = p3y.tile([P, 512], bf16, tag="ysb")
                        # alternate the PSUM->SBUF evacuation between DVE and ACT
                        if yi % 2 == 0:
                            nc.vector.tensor_copy(ysb[:], yps[:])
                        else:
                            nc.scalar.activation(ysb[:], yps[:], AF.Copy)
                        nc.sync.dma_start(
                            y[tb * P : (tb + 1) * P, nch * 512 : (nch + 1) * 512],
                            ysb[:],
                        )
                        yi += 1

                    for s in range(NSPAN):
                        q0 = s * SPAN
                        nkb = 4 * s + 4
                        ng = nkb // 2       # score/exp groups of 2 key-blocks
                        for h in range(NQ):
                            j = h // 2
                            ot_ps = ps_ot.tile([P, SPAN], f32, tag="ot_ps")
                            sum_ps = ps_nm.tile([1, SPAN], f32, tag="nm")
                            q_ap = qk_rt[:, h, q0 : q0 + SPAN]
                            for g in range(ng):
                                sc = ps_sc.tile([P, 2, SPAN], f32, tag="sc")
                                for jj in range(2):
                                    kb = 2 * g + jj
                                    nc.tensor.matmul(
                                        sc[:, jj, :],
                                        qk_rt[:, NQ + j, kb * P : (kb + 1) * P],
                                        q_ap,
                                        start=True, stop=True,
                                    )
                                pt = p3.tile([P, 2, SPAN], bf16, tag="pt")
                                nc.scalar.activation(
                                    pt[:, :, :], sc[:, :, :], AF.Exp, scale=SCALE
                                )
                                if g >= 2 * s:  # diagonal groups: causal mask
                                    r = 2 * g - 4 * s
                                    nc.vector.tensor_mul(
                                        pt[:, :, :], pt[:, :, :],
                                        mask_sb[:, r : r + 2, :],
                                    )
                                # AV accumulation
                                for jj in range(2):
                                    kb = 2 * g + jj
                                    nc.tensor.matmul(
                                        ot_ps[:],
                                        v_sb[:, kb, j * D : (j + 1) * D],
                                        pt[:, jj, :],
                                        start=(kb == 0), stop=(kb == nkb - 1),
                                        skip_group_check=True,
                                    )
                                # row-sum: pair-add on DVE, one matmul per group
                                lf = p3.tile([P, SPAN], bf16, tag="lf")
                                nc.vector.tensor_add(lf[:], pt[:, 0, :], pt[:, 1, :])
                                nc.tensor.matmul(
                                    sum_ps[:], ones_col[:], lf[:],
                                    start=(g == 0), stop=(g == ng - 1),
                                    skip_group_check=True,
                                )
                            # normalization: fast 1/sums on DVE, broadcast via PE
                            rec = p3.tile([1, SPAN], f32, tag="rec")
                            nc.vector.reciprocal_approx_fast(rec[:], sum_ps[:])
                            bc_ps = ps_nm.tile([P, SPAN], f32, tag="nm")
                            nc.tensor.matmul(
                                bc_ps[:], ones_row_f32[:], rec[:],
                                start=True, stop=True,
                            )
                            bc_sb = p3.tile([P, SPAN], f32, tag="bc_sb")
                            nc.scalar.activation(bc_sb[:], bc_ps[:], AF.Copy)
                            nc.vector.tensor_mul(
                                ot_sb[:, h, q0 : q0 + SPAN], ot_ps[:], bc_sb[:]
                            )
                            # output projection of the previous span, interleaved
                            # as PE filler while ACT crunches the next head's exps
                            if s >= 1:
                                tb = 4 * (s - 1) + h
                                for nch in range(C // 512):
                                    emit_proj_group(tb, nch)

                    # tail: projection for the last span
                    for tb in range(4 * (NSPAN - 1), 4 * NSPAN):
                        for nch in range(C // 512):
                            emit_proj_group(tb, nch)
    nc.compile()
    return nc


_NC_CACHE = None


def _get_nc():
    global _NC_CACHE
    if _NC_CACHE is None:
        _NC_CACHE = build()
    return _NC_CACHE


def _host_inputs(x, cos, sin, wq, wk, wv, wo):
    """Build the 8 per-core input maps."""
    bft = ml_dtypes.bfloat16
    cosT = np.ascontiguousarray(cos[0, :, 0, :].T).astype(np.float32)  # (64, T)
    sinT = np.ascontiguousarray(sin[0, :, 0, :].T).astype(np.float32)
    cc = np.concatenate([cosT, cosT], axis=0).astype(bft)          # (128, T)
    ss = np.concatenate([sinT, -sinT], axis=0).astype(bft)
    # maskT[r][k, q] = 1 if q >= 128*r + k  (within a 512-q span, k-block offset r)
    qidx = np.arange(SPAN)[None, None, :]
    kidx = np.arange(P)[:, None, None]
    ridx = np.arange(4)[None, :, None]
    maskT = (qidx >= P * ridx + kidx).astype(bft)  # (128, 4, 512)
    ident = np.eye(P, dtype=np.float32).astype(bft)

    xTs = [np.ascontiguousarray(x[b].T).astype(bft) for b in range(2)]
    wq16 = wq.astype(bft)
    wk16 = wk.astype(bft)
    wv16 = wv.astype(bft)
    wo16 = wo.astype(bft)
    in_maps = []
    for c in range(8):
        b, tp = divmod(c, 4)
        in_maps.append(
            {
                "xT": xTs[b],
                "wq": np.ascontiguousarray(wq16[:, tp * FQ : (tp + 1) * FQ]),
                "wk": np.ascontiguousarray(wk16[:, tp * FK : (tp + 1) * FK]),
                "wv": np.ascontiguousarray(wv16[:, tp * FK : (tp + 1) * FK]),
                "wo": np.ascontiguousarray(wo16[tp * FQ : (tp + 1) * FQ, :]),
                "cc": cc,
                "ss": ss,
                "maskT": maskT,
                "ident": ident,
            }
        )
    return in_maps


def kernel(x, cos, sin, wq, wk, wv, wo, trace=False):
    x = np.asarray(x, dtype=np.float32)
    cos = np.asarray(cos, dtype=np.float32)
    sin = np.asarray(sin, dtype=np.float32)
    wq = np.asarray(wq, dtype=np.float32)
    wk = np.asarray(wk, dtype=np.float32)
    wv = np.asarray(wv, dtype=np.float32)
    wo = np.asarray(wo, dtype=np.float32)

    nc = _get_nc()
    in_maps = _host_inputs(x, cos, sin, wq, wk, wv, wo)
    res = run_bass_kernel_spmd(nc, in_maps, core_ids=list(range(8)), trace=trace)
    out = np.zeros((2, T, C), dtype=np.float32)
    for c in range(8):
        b = c // 4
        out[b] += res.results[c]["y"].astype(np.float32)
    if trace:
        return out, res
    return out


# revision 12
# speedup vs baseline: 1.2418x; 1.0633x over previous
"""Causal self-attention (RoPE + QK-RMSNorm, GQA 16q/8kv) Trainium2 Bass kernel.

Sharding: 8 cores = 2 batch x 4 tensor-parallel. Core c handles batch b=c//4 and
q-heads [4*tp, 4*tp+4), kv-heads [2*tp, 2*tp+2) where tp=c%4. Each core returns a
partial (T, C) output = O_heads @ wo[rows of its heads]; host sums the 4 partials
per batch (the "all-reduce after c_proj").

Matmuls run in bf16 (fp32 PSUM accumulation); softmax row-sum normalization and
RMS statistics stay in fp32/fp32r.

Phase-3 structure: scores for 2 key-blocks land in one 2-bank PSUM tile, one exp
instruction covers both; row-sums use a DVE pair-add + one accumulating matmul
per group; projection matmuls of the previous span interleave into the current
span's attention stream to keep the PE dense (HAM stays at full clock).
"""
import sys
import math

sys.path.insert(0, "/opt/trn_rl_repo")

import numpy as np
import ml_dtypes
import concourse.bacc as bacc
import concourse.mybir as mybir
import concourse.tile as tile
from concourse.bass_utils import run_bass_kernel_spmd

P = 128
T = 2048
C = 2048
KO = C // P          # 16 contraction tiles
D = 128              # head dim
NQ = 4               # q heads per core
NK = 2               # kv heads per core
NF = NQ + NK         # 6 rope/rms feature blocks (4 q + 2 k)
FQ = NQ * D          # 512
FK = NK * D          # 256
TCH = 512            # phase-1 T-chunk
NCHUNK = T // TCH    # 4
SPAN = 512           # attention q-span
NSPAN = T // SPAN    # 4
KB = T // P          # 16 key blocks
SCALE = 1.0 / math.sqrt(D)
EPS = 1.1920929e-07

f32 = mybir.dt.float32
f32r = mybir.dt.float32r
bf16 = mybir.dt.bfloat16

AF = mybir.ActivationFunctionType


def build():
    nc = bacc.Bacc("TRN2", target_bir_lowering=False)
    xT = nc.dram_tensor("xT", (C, T), bf16, kind="ExternalInput")
    wq = nc.dram_tensor("wq", (C, FQ), bf16, kind="ExternalInput")
    wk = nc.dram_tensor("wk", (C, FK), bf16, kind="ExternalInput")
    wv = nc.dram_tensor("wv", (C, FK), bf16, kind="ExternalInput")
    wo = nc.dram_tensor("wo", (FQ, C), bf16, kind="ExternalInput")
    cc = nc.dram_tensor("cc", (P, T), bf16, kind="ExternalInput")    # [cos; cos]
    ss = nc.dram_tensor("ss", (P, T), bf16, kind="ExternalInput")    # [sin; -sin]
    maskT = nc.dram_tensor("maskT", (P, 4, SPAN), bf16, kind="ExternalInput")
    ident = nc.dram_tensor("ident", (P, P), bf16, kind="ExternalInput")
    y = nc.dram_tensor("y", (T, C), bf16, kind="ExternalOutput")

    xT_r = xT.rearrange("(ko p) t -> p ko t", p=P)
    wq_r = wq.rearrange("(ko p) f -> p ko f", p=P)
    wk_r = wk.rearrange("(ko p) f -> p ko f", p=P)
    wv_r = wv.rearrange("(ko p) f -> p ko f", p=P)
    wo_r = wo.rearrange("(ko p) n -> p ko n", p=P)

    with tile.TileContext(nc) as tc:
        with tc.tile_pool(name="persist", bufs=1) as persist:
            # persistent across phases
            qk_rt = persist.tile([P, NF, T], bf16, tag="qk_rt")   # roped+normed qT/kT
            v_sb = persist.tile([P, KB, FK], bf16, tag="v_sb")    # V natural [t-part, kb, feat]
            cc_sb = persist.tile([P, T], bf16, tag="cc_sb")
            ss_sb = persist.tile([P, T], bf16, tag="ss_sb")
            id_sb = persist.tile([P, P], bf16, tag="id_sb")
            ones_col = persist.tile([P, 1], bf16, tag="ones_col")    # sums lhsT
            ones_row = persist.tile([1, P], f32r, tag="ones_row")    # bcast lhsT
            eps_sb = persist.tile([P, 1], f32, tag="eps_sb")
            zero_sb = persist.tile([1, 1], f32, tag="zero_sb")
            nc.vector.memset(zero_sb[:], 0.0)
            ones_f32 = persist.tile([P, 1], f32, tag="ones_f32")
            ones_row_f32 = persist.tile([1, P], f32, tag="ones_row_f32")
            nc.vector.memset(eps_sb[:], EPS)
            nc.vector.memset(ones_f32[:], 1.0)
            nc.vector.memset(ones_row_f32[:], 1.0)
            nc.vector.tensor_copy(ones_col[:], ones_f32[:])
            nc.vector.tensor_copy(ones_row[:], ones_row_f32[:])

            # ------- Phase 1: QKV projections + RoPE + RMS norm + V transpose -------
            with (
                tc.tile_pool(name="ph1w", bufs=1) as wpool,
                tc.tile_pool(name="ph1x", bufs=2) as xpool,
                tc.tile_pool(name="ph1t", bufs=3) as tpool,
                tc.tile_pool(name="ph1ps", bufs=3, space="PSUM") as ps1,
                tc.tile_pool(name="ph1tr", bufs=2, space="PSUM") as pstr,
                tc.tile_pool(name="ph1ms", bufs=1, space="PSUM") as psms,
                tc.tile_pool(name="ph1rb", bufs=2, space="PSUM") as psrb,
            ):
                wq_sb = wpool.tile([P, KO, FQ], bf16, tag="wq_sb")
                wk_sb = wpool.tile([P, KO, FK], bf16, tag="wk_sb")
                wv_sb = wpool.tile([P, KO, FK], bf16, tag="wv_sb")
                # startup-critical DMA order: first chunk's x plus weights,
                # per-ko and interleaved in matmul consumption order (v first),
                # so the first matmul group can start after ~one ko-slice.
                xt0 = xpool.tile([P, KO, TCH], bf16, tag="xt", bufs=3)
                for ko in range(KO):
                    nc.sync.dma_start(xt0[:, ko, :], xT_r[:, ko, 0:TCH])
                    nc.sync.dma_start(wv_sb[:, ko, :], wv_r[:, ko, :])
                    nc.sync.dma_start(wq_sb[:, ko, :], wq_r[:, ko, :])
                    nc.sync.dma_start(wk_sb[:, ko, :], wk_r[:, ko, :])
                nc.sync.dma_start(id_sb[:], ident[:, :])
                nc.sync.dma_start(cc_sb[:], cc[:, :])
                nc.sync.dma_start(ss_sb[:], ss[:, :])

                # Software-pipelined over feature blocks, carried across chunk
                # boundaries: each block's rope/RMS/transpose post-processing is
                # deferred by 2 matmul groups (~7us of PE cover) so the in-order
                # DVE queue and the swap DMAs never stall the PE. The RMS
                # broadcast+apply trails one further post-step.
                posts = []   # deferred post-process closures
                appls = []   # deferred RMS broadcast+apply

                def emit_post_qk(args):
                    fb, tch, raw, swp = args
                    t0 = tch * TCH
                    tmpa = tpool.tile([P, TCH], bf16, tag="rope_tmpa")
                    tmpb = tpool.tile([P, TCH], bf16, tag="rope_tmpb")
                    seg = qk_rt[:, fb, t0 : t0 + TCH]
                    nc.vector.tensor_mul(tmpa[:], raw[:], cc_sb[:, t0 : t0 + TCH])
                    nc.vector.tensor_mul(tmpb[:], swp[:], ss_sb[:, t0 : t0 + TCH])
                    nc.vector.tensor_add(seg, tmpa[:], tmpb[:])
                    # RMS stats: sum of squares over head dim (partitions)
                    sq = tpool.tile([P, TCH], bf16, tag="sq")
                    nc.vector.tensor_mul(sq[:], seg, seg)
                    pms = psms.tile([1, TCH], f32, tag="ps_ms")
                    nc.tensor.matmul(pms[:], ones_col[:], sq[:], start=True, stop=True)
                    # rstd = 1/sqrt(ms/D + eps): ACT Sqrt + fast DVE recip.
                    # (Ln+Exp here would thrash ACT table sets against the
                    # attention phase's Exp — 1.28us per switch.)
                    sqms = tpool.tile([1, TCH], f32, tag="sqms", bufs=5)
                    nc.scalar.activation(
                        sqms[:], pms[:], AF.Sqrt, bias=eps_sb[0:1, :], scale=1.0 / D
                    )
                    rstd_raw = tpool.tile([1, TCH], f32, tag="rstd_raw", bufs=5)
                    nc.vector.reciprocal_approx_fast(rstd_raw[:], sqms[:])
                    # f32r for a single-pass fp32 broadcast matmul (plain f32
                    # lhsT/rhs lowers to two half-rate MATMUL passes)
                    rstd = tpool.tile([1, TCH], f32r, tag="rstd", bufs=5)
                    nc.scalar.activation(rstd[:], rstd_raw[:], AF.Copy)
                    appls.append((seg, rstd))

                def emit_post_v(args):
                    vfb, tch, vt_sb = args
                    ptr = pstr.tile([P, TCH // P, P], bf16, tag="ps_tr")
                    for tb in range(TCH // P):
                        nc.tensor.transpose(
                            ptr[:, tb, :],
                            vt_sb[:, tb * P : (tb + 1) * P],
                            id_sb[:],
                        )
                    nc.vector.tensor_copy(
                        v_sb[
                            :,
                            tch * (TCH // P) : (tch + 1) * (TCH // P),
                            vfb * D : (vfb + 1) * D,
                        ],
                        ptr[:, :, :],
                    )

                def drain_one_post():
                    kind, args = posts.pop(0)
                    (emit_post_qk if kind == "qk" else emit_post_v)(args)
                    if len(appls) > 1:
                        seg, rstd = appls.pop(0)
                        pb = psrb.tile([P, TCH], f32, tag="ps_b")
                        nc.tensor.matmul(
                            pb[:], ones_row[:], rstd[:], start=True, stop=True
                        )
                        nc.vector.tensor_mul(seg, seg, pb[:])

                for tch in range(NCHUNK):
                    t0 = tch * TCH
                    if tch == 0:
                        xt = xt0
                    else:
                        xt = xpool.tile([P, KO, TCH], bf16, tag="xt", bufs=3)
                        for ko in range(KO):
                            nc.sync.dma_start(xt[:, ko, :], xT_r[:, ko, t0 : t0 + TCH])
                    # v first: its post (PE transposes) is cheap and swap-free
                    for vfb in range(NK):
                        w_ap = wv_sb[:, :, vfb * D : (vfb + 1) * D]
                        pvt = ps1.tile([P, TCH], f32, tag="ps_qkv")
                        for ko in range(KO):
                            nc.tensor.matmul(
                                pvt[:], w_ap[:, ko], xt[:, ko, :],
                                start=(ko == 0), stop=(ko == KO - 1),
                            )
                        vt_sb = tpool.tile([P, TCH], bf16, tag="vt_sb", bufs=3)
                        nc.vector.tensor_copy(vt_sb[:], pvt[:])
                        posts.append(("v", (vfb, tch, vt_sb)))
                        if len(posts) > 2:
                            drain_one_post()
                    for fb in range(NF):
                        if fb < NQ:
                            w_ap = wq_sb[:, :, fb * D : (fb + 1) * D]
                        else:
                            w_ap = wk_sb[:, :, (fb - NQ) * D : (fb - NQ + 1) * D]
                        pqk = ps1.tile([P, TCH], f32, tag="ps_qkv")
                        for ko in range(KO):
                            nc.tensor.matmul(
                                pqk[:], w_ap[:, ko], xt[:, ko, :],
                                start=(ko == 0), stop=(ko == KO - 1),
                            )
                        raw = tpool.tile([P, TCH], bf16, tag="rope_raw", bufs=5)
                        nc.vector.tensor_copy(raw[:], pqk[:])
                        swp = tpool.tile([P, TCH], bf16, tag="rope_swp", bufs=5)
                        nc.sync.dma_start(swp[0:64, :], raw[64:128, :])
                        nc.sync.dma_start(swp[64:128, :], raw[0:64, :])
                        posts.append(("qk", (fb, tch, raw, swp)))
                        if len(posts) > 2:
                            drain_one_post()
                while posts:
                    drain_one_post()
                while appls:
                    seg, rstd = appls.pop(0)
                    pb = psrb.tile([P, TCH], f32, tag="ps_b")
                    nc.tensor.matmul(
                        pb[:], ones_row[:], rstd[:], start=True, stop=True
                    )
                    nc.vector.tensor_mul(seg, seg, pb[:])

            # ---------------- Phase 3: attention + Phase 4: output projection ------------
            with (
                tc.tile_pool(name="ph3s", bufs=1) as p3s,
                tc.tile_pool(name="ph3t", bufs=3) as p3,
                tc.tile_pool(name="ph3y", bufs=3) as p3y,
            ):
                ot_sb = p3s.tile([P, NQ, T], bf16, tag="ot_sb")
                mask_sb = p3s.tile([P, 4, SPAN], bf16, tag="mask_sb")
                wo_sb = p3s.tile([P, NQ, C], bf16, tag="wo_sb")
                nc.sync.dma_start(mask_sb[:], maskT[:, :, :])
                for ko in range(NQ):
                    nc.sync.dma_start(wo_sb[:, ko, :], wo_r[:, ko, :])

                with (
                    tc.tile_pool(name="ph3sc", bufs=2, space="PSUM") as ps_sc,
                    tc.tile_pool(name="ph3ot", bufs=1, space="PSUM") as ps_ot,
                    tc.tile_pool(name="ph3nm", bufs=1, space="PSUM") as ps_nm,
                    tc.tile_pool(name="ph3yp", bufs=2, space="PSUM") as ps_yp,
                ):
                    yi = 0  # global proj-group counter (for engine alternation)

                    def emit_proj_group(tb, nch):
                        nonlocal yi
                        yps = ps_yp.tile([P, 512], f32, tag="yps")
                        for hh in range(NQ):
                            nc.tensor.matmul(
                                yps[:],
                                ot_sb[:, hh, tb * P : (tb + 1) * P],
                                wo_sb[:, hh, nch * 512 : (nch + 1) * 512],
                                start=(hh == 0), stop=(hh == NQ - 1),
                            )
                        ysb = p3y.tile([P, 512], bf16, tag="ysb")
                        # alternate the PSUM->SBUF evacuation between DVE and ACT
                        if yi % 2 == 0:
                            nc.vector.tensor_copy(ysb[:], yps[:])
                        else:
                            nc.scalar.activation(ysb[:], yps[:], AF.Copy)
                        nc.sync.dma_start(
                            y[tb * P : (tb + 1) * P, nch * 512 : (nch + 1) * 512],
                            ysb[:],
                        )
                        yi += 1

                    for s in range(NSPAN):
                        q0 = s * SPAN
                        nkb = 4 * s + 4
                        ng = nkb // 2       # score/exp groups of 2 key-blocks
                        for h in range(NQ):
                            j = h // 2
                            ot_ps = ps_ot.tile([P, SPAN], f32, tag="ot_ps")
                            sum_ps = ps_nm.tile([1, SPAN], f32, tag="nm")
                            q_ap = qk_rt[:, h, q0 : q0 + SPAN]
                            for g in range(ng):
                                sc = ps_sc.tile([P, 2, SPAN], f32, tag="sc")
                                for jj in range(2):
                                    kb = 2 * g + jj
                                    nc.tensor.matmul(
                                        sc[:, jj, :],
                                        qk_rt[:, NQ + j, kb * P : (kb + 1) * P],
                                        q_ap,
                                        start=True, stop=True,
                                    )
                                pt = p3.tile([P, 2, SPAN], bf16, tag="pt")
                                nc.scalar.activation(
                                    pt[:, :, :], sc[:, :, :], AF.Exp, scale=SCALE
                                )
                                if g >= 2 * s:  # diagonal groups: causal mask
                                    r = 2 * g - 4 * s
                                    nc.vector.tensor_mul(
                                        pt[:, :, :], pt[:, :, :],
                                        mask_sb[:, r : r + 2, :],
                                    )
                                # AV accumulation
                                for jj in range(2):
                                    kb = 2 * g + jj
                                    nc.tensor.matmul(
                                        ot_ps[:],
                                        v_sb[:, kb, j * D : (j + 1) * D],
                                        pt[:, jj, :],
                                        start=(kb == 0), stop=(kb == nkb - 1),
                                        skip_group_check=True,
                                    )
                                # row-sum: pair-add on DVE, one matmul per group
                                lf = p3.tile([P, SPAN], bf16, tag="lf")
                                nc.vector.tensor_add(lf[:], pt[:, 0, :], pt[:, 1, :])
                                nc.tensor.matmul(
                                    sum_ps[:], ones_col[:], lf[:],
                                    start=(g == 0), stop=(g == ng - 1),
                                    skip_group_check=True,
                                )
                            # normalization: fast 1/sums on DVE, broadcast via PE
                            rec_raw = p3.tile([1, SPAN], f32, tag="rec_raw")
                            nc.vector.reciprocal_approx_fast(rec_raw[:], sum_ps[:])
                            rec = p3.tile([1, SPAN], f32r, tag="rec")
                            nc.scalar.activation(rec[:], rec_raw[:], AF.Copy)
                            bc_ps = ps_nm.tile([P, SPAN], f32, tag="nm")
                            nc.tensor.matmul(
                                bc_ps[:], ones_row[:], rec[:],
                                start=True, stop=True,
                            )
                            bc_sb = p3.tile([P, SPAN], f32, tag="bc_sb")
                            nc.scalar.activation(bc_sb[:], bc_ps[:], AF.Copy)
                            nc.vector.tensor_mul(
                                ot_sb[:, h, q0 : q0 + SPAN], ot_ps[:], bc_sb[:]
                            )
                            # output projection of the previous span, interleaved
                            # as PE filler while ACT crunches the next head's exps
                            if s >= 1:
                                tb = 4 * (s - 1) + h
                                for nch in range(C // 512):
                                    emit_proj_group(tb, nch)

                    # tail: projection for the last span
                    for tb in range(4 * (NSPAN - 1), 4 * NSPAN):
                        for nch in range(C // 512):
                            emit_proj_group(tb, nch)
    nc.compile()
    return nc


_NC_CACHE = None


def _get_nc():
    global _NC_CACHE
    if _NC_CACHE is None:
        _NC_CACHE = build()
    return _NC_CACHE


def _host_inputs(x, cos, sin, wq, wk, wv, wo):
    """Build the 8 per-core input maps."""
    bft = ml_dtypes.bfloat16
    cosT = np.ascontiguousarray(cos[0, :, 0, :].T).astype(np.float32)  # (64, T)
    sinT = np.ascontiguousarray(sin[0, :, 0, :].T).astype(np.float32)
    cc = np.concatenate([cosT, cosT], axis=0).astype(bft)          # (128, T)
    ss = np.concatenate([sinT, -sinT], axis=0).astype(bft)
    # maskT[r][k, q] = 1 if q >= 128*r + k  (within a 512-q span, k-block offset r)
    qidx = np.arange(SPAN)[None, None, :]
    kidx = np.arange(P)[:, None, None]
    ridx = np.arange(4)[None, :, None]
    maskT = (qidx >= P * ridx + kidx).astype(bft)  # (128, 4, 512)
    ident = np.eye(P, dtype=np.float32).astype(bft)

    xTs = [np.ascontiguousarray(x[b].T).astype(bft) for b in range(2)]
    wq16 = wq.astype(bft)
    wk16 = wk.astype(bft)
    wv16 = wv.astype(bft)
    wo16 = wo.astype(bft)
    in_maps = []
    for c in range(8):
        b, tp = divmod(c, 4)
        in_maps.append(
            {
                "xT": xTs[b],
                "wq": np.ascontiguousarray(wq16[:, tp * FQ : (tp + 1) * FQ]),
                "wk": np.ascontiguousarray(wk16[:, tp * FK : (tp + 1) * FK]),
                "wv": np.ascontiguousarray(wv16[:, tp * FK : (tp + 1) * FK]),
                "wo": np.ascontiguousarray(wo16[tp * FQ : (tp + 1) * FQ, :]),
                "cc": cc,
                "ss": ss,
                "maskT": maskT,
                "ident": ident,
            }
        )
    return in_maps


def kernel(x, cos, sin, wq, wk, wv, wo, trace=False):
    x = np.asarray(x, dtype=np.float32)
    cos = np.asarray(cos, dtype=np.float32)
    sin = np.asarray(sin, dtype=np.float32)
    wq = np.asarray(wq, dtype=np.float32)
    wk = np.asarray(wk, dtype=np.float32)
    wv = np.asarray(wv, dtype=np.float32)
    wo = np.asarray(wo, dtype=np.float32)

    nc = _get_nc()
    in_maps = _host_inputs(x, cos, sin, wq, wk, wv, wo)
    res = run_bass_kernel_spmd(nc, in_maps, core_ids=list(range(8)), trace=trace)
    out = np.zeros((2, T, C), dtype=np.float32)
    for c in range(8):
        b = c // 4
        out[b] += res.results[c]["y"].astype(np.float32)
    if trace:
        return out, res
    return out


# revision 14
# speedup vs baseline: 1.2425x; 1.0005x over previous
"""Causal self-attention (RoPE + QK-RMSNorm, GQA 16q/8kv) Trainium2 Bass kernel.

Sharding: 8 cores = 2 batch x 4 tensor-parallel. Core c handles batch b=c//4 and
q-heads [4*tp, 4*tp+4), kv-heads [2*tp, 2*tp+2) where tp=c%4. Each core returns a
partial (T, C) output = O_heads @ wo[rows of its heads]; host sums the 4 partials
per batch (the "all-reduce after c_proj").

Matmuls run in bf16 (fp32 PSUM accumulation); softmax row-sum normalization and
RMS statistics stay in fp32/fp32r.

Phase-3 structure: scores for 2 key-blocks land in one 2-bank PSUM tile, one exp
instruction covers both; row-sums use a DVE pair-add + one accumulating matmul
per group; projection matmuls of the previous span interleave into the current
span's attention stream to keep the PE dense (HAM stays at full clock).
"""
import sys
import math

sys.path.insert(0, "/opt/trn_rl_repo")

import numpy as np
import ml_dtypes
import concourse.bacc as bacc
import concourse.mybir as mybir
import concourse.tile as tile
from concourse.bass_utils import run_bass_kernel_spmd

P = 128
T = 2048
C = 2048
KO = C // P          # 16 contraction tiles
D = 128              # head dim
NQ = 4               # q heads per core
NK = 2               # kv heads per core
NF = NQ + NK         # 6 rope/rms feature blocks (4 q + 2 k)
FQ = NQ * D          # 512
FK = NK * D          # 256
TCH = 512            # phase-1 T-chunk
NCHUNK = T // TCH    # 4
SPAN = 512           # attention q-span
NSPAN = T // SPAN    # 4
KB = T // P          # 16 key blocks
SCALE = 1.0 / math.sqrt(D)
EPS = 1.1920929e-07

f32 = mybir.dt.float32
f32r = mybir.dt.float32r
bf16 = mybir.dt.bfloat16

AF = mybir.ActivationFunctionType


def build():
    nc = bacc.Bacc("TRN2", target_bir_lowering=False)
    xT = nc.dram_tensor("xT", (C, T), bf16, kind="ExternalInput")
    wq = nc.dram_tensor("wq", (C, FQ), bf16, kind="ExternalInput")
    wk = nc.dram_tensor("wk", (C, FK), bf16, kind="ExternalInput")
    wv = nc.dram_tensor("wv", (C, FK), bf16, kind="ExternalInput")
    wo = nc.dram_tensor("wo", (FQ, C), bf16, kind="ExternalInput")
    cc = nc.dram_tensor("cc", (P, T), bf16, kind="ExternalInput")    # [cos; cos]
    ss = nc.dram_tensor("ss", (P, T), bf16, kind="ExternalInput")    # [sin; -sin]
    maskT = nc.dram_tensor("maskT", (P, 4, SPAN), bf16, kind="ExternalInput")
    ident = nc.dram_tensor("ident", (P, P), bf16, kind="ExternalInput")
    y = nc.dram_tensor("y", (T, C), bf16, kind="ExternalOutput")

    xT_r = xT.rearrange("(ko p) t -> p ko t", p=P)
    wq_r = wq.rearrange("(ko p) f -> p ko f", p=P)
    wk_r = wk.rearrange("(ko p) f -> p ko f", p=P)
    wv_r = wv.rearrange("(ko p) f -> p ko f", p=P)
    wo_r = wo.rearrange("(ko p) n -> p ko n", p=P)

    with tile.TileContext(nc) as tc:
        with tc.tile_pool(name="persist", bufs=1) as persist:
            # persistent across phases
            qk_rt = persist.tile([P, NF, T], bf16, tag="qk_rt")   # roped+normed qT/kT
            v_sb = persist.tile([P, KB, FK], bf16, tag="v_sb")    # V natural [t-part, kb, feat]
            cc_sb = persist.tile([P, T], bf16, tag="cc_sb")
            ss_sb = persist.tile([P, T], bf16, tag="ss_sb")
            id_sb = persist.tile([P, P], bf16, tag="id_sb")
            ones_col = persist.tile([P, 1], bf16, tag="ones_col")    # sums lhsT
            ones_row = persist.tile([1, P], f32r, tag="ones_row")    # bcast lhsT
            eps_sb = persist.tile([P, 1], f32, tag="eps_sb")
            zero_sb = persist.tile([1, 1], f32, tag="zero_sb")
            nc.vector.memset(zero_sb[:], 0.0)
            ones_f32 = persist.tile([P, 1], f32, tag="ones_f32")
            ones_row_f32 = persist.tile([1, P], f32, tag="ones_row_f32")
            nc.vector.memset(eps_sb[:], EPS)
            nc.vector.memset(ones_f32[:], 1.0)
            nc.vector.memset(ones_row_f32[:], 1.0)
            nc.vector.tensor_copy(ones_col[:], ones_f32[:])
            nc.vector.tensor_copy(ones_row[:], ones_row_f32[:])

            # ------- Phase 1: QKV projections + RoPE + RMS norm + V transpose -------
            with (
                tc.tile_pool(name="ph1w", bufs=1) as wpool,
                tc.tile_pool(name="ph1x", bufs=2) as xpool,
                tc.tile_pool(name="ph1t", bufs=3) as tpool,
                tc.tile_pool(name="ph1ps", bufs=3, space="PSUM") as ps1,
                tc.tile_pool(name="ph1tr", bufs=2, space="PSUM") as pstr,
                tc.tile_pool(name="ph1ms", bufs=1, space="PSUM") as psms,
                tc.tile_pool(name="ph1rb", bufs=2, space="PSUM") as psrb,
            ):
                wq_sb = wpool.tile([P, KO, FQ], bf16, tag="wq_sb")
                wk_sb = wpool.tile([P, KO, FK], bf16, tag="wk_sb")
                wv_sb = wpool.tile([P, KO, FK], bf16, tag="wv_sb")
                # startup-critical DMA order: first chunk's x plus weights,
                # per-ko and interleaved in matmul consumption order (v first),
                # so the first matmul group can start after ~one ko-slice.
                xt0 = xpool.tile([P, KO, TCH], bf16, tag="xt", bufs=3)
                for ko in range(KO):
                    nc.sync.dma_start(xt0[:, ko, :], xT_r[:, ko, 0:TCH])
                    nc.sync.dma_start(wv_sb[:, ko, :], wv_r[:, ko, :])
                    nc.sync.dma_start(wq_sb[:, ko, :], wq_r[:, ko, :])
                    nc.sync.dma_start(wk_sb[:, ko, :], wk_r[:, ko, :])
                    if ko == 0:
                        nc.sync.dma_start(id_sb[:], ident[:, :])
                    if ko == 2:  # needed by the first rope post (~10us in)
                        nc.sync.dma_start(cc_sb[:], cc[:, :])
                        nc.sync.dma_start(ss_sb[:], ss[:, :])

                # Software-pipelined over feature blocks, carried across chunk
                # boundaries: each block's rope/RMS/transpose post-processing is
                # deferred by 2 matmul groups (~7us of PE cover) so the in-order
                # DVE queue and the swap DMAs never stall the PE. The RMS
                # broadcast+apply trails one further post-step.
                posts = []   # deferred post-process closures
                appls = []   # deferred RMS broadcast+apply

                def emit_post_qk(args):
                    fb, tch, raw, swp = args
                    t0 = tch * TCH
                    tmpa = tpool.tile([P, TCH], bf16, tag="rope_tmpa")
                    tmpb = tpool.tile([P, TCH], bf16, tag="rope_tmpb")
                    seg = qk_rt[:, fb, t0 : t0 + TCH]
                    nc.vector.tensor_mul(tmpa[:], raw[:], cc_sb[:, t0 : t0 + TCH])
                    nc.vector.tensor_mul(tmpb[:], swp[:], ss_sb[:, t0 : t0 + TCH])
                    nc.vector.tensor_add(seg, tmpa[:], tmpb[:])
                    # RMS stats: sum of squares over head dim (partitions)
                    sq = tpool.tile([P, TCH], bf16, tag="sq")
                    nc.vector.tensor_mul(sq[:], seg, seg)
                    pms = psms.tile([1, TCH], f32, tag="ps_ms")
                    nc.tensor.matmul(pms[:], ones_col[:], sq[:], start=True, stop=True)
                    # rstd = 1/sqrt(ms/D + eps): ACT Sqrt + fast DVE recip.
                    # (Ln+Exp here would thrash ACT table sets against the
                    # attention phase's Exp — 1.28us per switch.)
                    sqms = tpool.tile([1, TCH], f32, tag="sqms", bufs=5)
                    nc.scalar.activation(
                        sqms[:], pms[:], AF.Sqrt, bias=eps_sb[0:1, :], scale=1.0 / D
                    )
                    rstd_raw = tpool.tile([1, TCH], f32, tag="rstd_raw", bufs=5)
                    nc.vector.reciprocal_approx_fast(rstd_raw[:], sqms[:])
                    # f32r for a single-pass fp32 broadcast matmul (plain f32
                    # lhsT/rhs lowers to two half-rate MATMUL passes)
                    rstd = tpool.tile([1, TCH], f32r, tag="rstd", bufs=5)
                    nc.scalar.activation(rstd[:], rstd_raw[:], AF.Copy)
                    appls.append((seg, rstd))

                def emit_post_v(args):
                    vfb, tch, vt_sb = args
                    ptr = pstr.tile([P, TCH // P, P], bf16, tag="ps_tr")
                    for tb in range(TCH // P):
                        nc.tensor.transpose(
                            ptr[:, tb, :],
                            vt_sb[:, tb * P : (tb + 1) * P],
                            id_sb[:],
                        )
                    nc.vector.tensor_copy(
                        v_sb[
                            :,
                            tch * (TCH // P) : (tch + 1) * (TCH // P),
                            vfb * D : (vfb + 1) * D,
                        ],
                        ptr[:, :, :],
                    )

                def drain_one_post():
                    kind, args = posts.pop(0)
                    (emit_post_qk if kind == "qk" else emit_post_v)(args)
                    if len(appls) > 1:
                        seg, rstd = appls.pop(0)
                        pb = psrb.tile([P, TCH], f32, tag="ps_b")
                        nc.tensor.matmul(
                            pb[:], ones_row[:], rstd[:], start=True, stop=True
                        )
                        nc.vector.tensor_mul(seg, seg, pb[:])

                for tch in range(NCHUNK):
                    t0 = tch * TCH
                    if tch == 0:
                        xt = xt0
                    else:
                        xt = xpool.tile([P, KO, TCH], bf16, tag="xt", bufs=3)
                        for ko in range(KO):
                            nc.sync.dma_start(xt[:, ko, :], xT_r[:, ko, t0 : t0 + TCH])
                    # v first: its post (PE transposes) is cheap and swap-free
                    for vfb in range(NK):
                        w_ap = wv_sb[:, :, vfb * D : (vfb + 1) * D]
                        pvt = ps1.tile([P, TCH], f32, tag="ps_qkv")
                        for ko in range(KO):
                            nc.tensor.matmul(
                                pvt[:], w_ap[:, ko], xt[:, ko, :],
                                start=(ko == 0), stop=(ko == KO - 1),
                            )
                        vt_sb = tpool.tile([P, TCH], bf16, tag="vt_sb", bufs=3)
                        nc.vector.tensor_copy(vt_sb[:], pvt[:])
                        posts.append(("v", (vfb, tch, vt_sb)))
                        if len(posts) > 2:
                            drain_one_post()
                    for fb in range(NF):
                        if fb < NQ:
                            w_ap = wq_sb[:, :, fb * D : (fb + 1) * D]
                        else:
                            w_ap = wk_sb[:, :, (fb - NQ) * D : (fb - NQ + 1) * D]
                        pqk = ps1.tile([P, TCH], f32, tag="ps_qkv")
                        for ko in range(KO):
                            nc.tensor.matmul(
                                pqk[:], w_ap[:, ko], xt[:, ko, :],
                                start=(ko == 0), stop=(ko == KO - 1),
                            )
                        raw = tpool.tile([P, TCH], bf16, tag="rope_raw", bufs=5)
                        nc.vector.tensor_copy(raw[:], pqk[:])
                        swp = tpool.tile([P, TCH], bf16, tag="rope_swp", bufs=5)
                        nc.sync.dma_start(swp[0:64, :], raw[64:128, :])
                        nc.sync.dma_start(swp[64:128, :], raw[0:64, :])
                        posts.append(("qk", (fb, tch, raw, swp)))
                        if len(posts) > 2:
                            drain_one_post()
                while posts:
                    drain_one_post()
                while appls:
                    seg, rstd = appls.pop(0)
                    pb = psrb.tile([P, TCH], f32, tag="ps_b")
                    nc.tensor.matmul(
                        pb[:], ones_row[:], rstd[:], start=True, stop=True
                    )
                    nc.vector.tensor_mul(seg, seg, pb[:])

            # ---------------- Phase 3: attention + Phase 4: output projection ------------
            with (
                tc.tile_pool(name="ph3s", bufs=1) as p3s,
                tc.tile_pool(name="ph3t", bufs=3) as p3,
                tc.tile_pool(name="ph3y", bufs=3) as p3y,
            ):
                ot_sb = p3s.tile([P, NQ, T], bf16, tag="ot_sb")
                mask_sb = p3s.tile([P, 4, SPAN], bf16, tag="mask_sb")
                wo_sb = p3s.tile([P, NQ, C], bf16, tag="wo_sb")
                nc.sync.dma_start(mask_sb[:], maskT[:, :, :])
                for ko in range(NQ):
                    nc.sync.dma_start(wo_sb[:, ko, :], wo_r[:, ko, :])

                with (
                    tc.tile_pool(name="ph3sc", bufs=2, space="PSUM") as ps_sc,
                    tc.tile_pool(name="ph3ot", bufs=1, space="PSUM") as ps_ot,
                    tc.tile_pool(name="ph3nm", bufs=1, space="PSUM") as ps_nm,
                    tc.tile_pool(name="ph3yp", bufs=2, space="PSUM") as ps_yp,
                ):
                    yi = 0  # global proj-group counter (for engine alternation)

                    def emit_proj_group(tb, nch):
                        nonlocal yi
                        yps = ps_yp.tile([P, 512], f32, tag="yps")
                        for hh in range(NQ):
                            nc.tensor.matmul(
                                yps[:],
                                ot_sb[:, hh, tb * P : (tb + 1) * P],
                                wo_sb[:, hh, nch * 512 : (nch + 1) * 512],
                                start=(hh == 0), stop=(hh == NQ - 1),
                            )
                        ysb = p3y.tile([P, 512], bf16, tag="ysb")
                        # alternate the PSUM->SBUF evacuation between DVE and ACT
                        if yi % 2 == 0:
                            nc.vector.tensor_copy(ysb[:], yps[:])
                        else:
                            nc.scalar.activation(ysb[:], yps[:], AF.Copy)
                        nc.sync.dma_start(
                            y[tb * P : (tb + 1) * P, nch * 512 : (nch + 1) * 512],
                            ysb[:],
                        )
                        yi += 1

                    # FIFO of the previous span's projection groups; spent as PE
                    # filler at the per-head stall points (softmax-normalize
                    # chain latency, exp pipeline refill).
                    proj_fifo = []

                    def proj_fill(n):
                        for _ in range(min(n, len(proj_fifo))):
                            tb, nch = proj_fifo.pop(0)
                            emit_proj_group(tb, nch)

                    for s in range(NSPAN):
                        q0 = s * SPAN
                        nkb = 4 * s + 4
                        ng = nkb // 2       # score/exp groups of 2 key-blocks
                        for h in range(NQ):
                            j = h // 2
                            ot_ps = ps_ot.tile([P, SPAN], f32, tag="ot_ps")
                            sum_ps = ps_nm.tile([1, SPAN], f32, tag="nm")
                            q_ap = qk_rt[:, h, q0 : q0 + SPAN]
                            for g in range(ng):
                                sc = ps_sc.tile([P, 2, SPAN], f32, tag="sc")
                                for jj in range(2):
                                    kb = 2 * g + jj
                                    nc.tensor.matmul(
                                        sc[:, jj, :],
                                        qk_rt[:, NQ + j, kb * P : (kb + 1) * P],
                                        q_ap,
                                        start=True, stop=True,
                                    )
                                pt = p3.tile([P, 2, SPAN], bf16, tag="pt")
                                nc.scalar.activation(
                                    pt[:, :, :], sc[:, :, :], AF.Exp, scale=SCALE
                                )
                                if g == 0:
                                    # cover the exp latency before the first AV
                                    proj_fill(2)
                                if g >= 2 * s:  # diagonal groups: causal mask
                                    r = 2 * g - 4 * s
                                    nc.vector.tensor_mul(
                                        pt[:, :, :], pt[:, :, :],
                                        mask_sb[:, r : r + 2, :],
                                    )
                                # AV accumulation
                                for jj in range(2):
                                    kb = 2 * g + jj
                                    nc.tensor.matmul(
                                        ot_ps[:],
                                        v_sb[:, kb, j * D : (j + 1) * D],
                                        pt[:, jj, :],
                                        start=(kb == 0), stop=(kb == nkb - 1),
                                        skip_group_check=True,
                                    )
                                # row-sum: pair-add on DVE, one matmul per group
                                lf = p3.tile([P, SPAN], bf16, tag="lf")
                                nc.vector.tensor_add(lf[:], pt[:, 0, :], pt[:, 1, :])
                                nc.tensor.matmul(
                                    sum_ps[:], ones_col[:], lf[:],
                                    start=(g == 0), stop=(g == ng - 1),
                                    skip_group_check=True,
                                )
                            # normalization: fast 1/sums + f32r copy on DVE (off
                            # the ACT exp queue), broadcast via PE
                            rec_raw = p3.tile([1, SPAN], f32, tag="rec_raw")
                            nc.vector.reciprocal_approx_fast(rec_raw[:], sum_ps[:])
                            rec = p3.tile([1, SPAN], f32r, tag="rec")
                            nc.vector.tensor_copy(rec[:], rec_raw[:])
                            proj_fill(1)  # cover the reciprocal chain latency
                            bc_ps = ps_nm.tile([P, SPAN], f32, tag="nm")
                            nc.tensor.matmul(
                                bc_ps[:], ones_row[:], rec[:],
                                start=True, stop=True,
                            )
                            bc_sb = p3.tile([P, SPAN], f32, tag="bc_sb")
                            nc.vector.tensor_copy(bc_sb[:], bc_ps[:])
                            nc.vector.tensor_mul(
                                ot_sb[:, h, q0 : q0 + SPAN], ot_ps[:], bc_sb[:]
                            )
                            proj_fill(1)
                        # refill the FIFO with this span's projection work for
                        # the next span (plus any leftovers)
                        for tb in range(4 * s, 4 * s + 4):
                            for nch in range(C // 512):
                                proj_fifo.append((tb, nch))

                    # tail: drain the last span's projection
                    while proj_fifo:
                        tb, nch = proj_fifo.pop(0)
                        emit_proj_group(tb, nch)
    nc.compile()
    return nc


_NC_CACHE = None


def _get_nc():
    global _NC_CACHE
    if _NC_CACHE is None:
        _NC_CACHE = build()
    return _NC_CACHE


def _host_inputs(x, cos, sin, wq, wk, wv, wo):
    """Build the 8 per-core input maps."""
    bft = ml_dtypes.bfloat16
    cosT = np.ascontiguousarray(cos[0, :, 0, :].T).astype(np.float32)  # (64, T)
    sinT = np.ascontiguousarray(sin[0, :, 0, :].T).astype(np.float32)
    cc = np.concatenate([cosT, cosT], axis=0).astype(bft)          # (128, T)
    ss = np.concatenate([sinT, -sinT], axis=0).astype(bft)
    # maskT[r][k, q] = 1 if q >= 128*r + k  (within a 512-q span, k-block offset r)
    qidx = np.arange(SPAN)[None, None, :]
    kidx = np.arange(P)[:, None, None]
    ridx = np.arange(4)[None, :, None]
    maskT = (qidx >= P * ridx + kidx).astype(bft)  # (128, 4, 512)
    ident = np.eye(P, dtype=np.float32).astype(bft)

    xTs = [np.ascontiguousarray(x[b].T).astype(bft) for b in range(2)]
    wq16 = wq.astype(bft)
    wk16 = wk.astype(bft)
    wv16 = wv.astype(bft)
    wo16 = wo.astype(bft)
    in_maps = []
    for c in range(8):
        b, tp = divmod(c, 4)
        in_maps.append(
            {
                "xT": xTs[b],
                "wq": np.ascontiguousarray(wq16[:, tp * FQ : (tp + 1) * FQ]),
                "wk": np.ascontiguousarray(wk16[:, tp * FK : (tp + 1) * FK]),
                "wv": np.ascontiguousarray(wv16[:, tp * FK : (tp + 1) * FK]),
                "wo": np.ascontiguousarray(wo16[tp * FQ : (tp + 1) * FQ, :]),
                "cc": cc,
                "ss": ss,
                "maskT": maskT,
                "ident": ident,
            }
        )
    return in_maps


def kernel(x, cos, sin, wq, wk, wv, wo, trace=False):
    x = np.asarray(x, dtype=np.float32)
    cos = np.asarray(cos, dtype=np.float32)
    sin = np.asarray(sin, dtype=np.float32)
    wq = np.asarray(wq, dtype=np.float32)
    wk = np.asarray(wk, dtype=np.float32)
    wv = np.asarray(wv, dtype=np.float32)
    wo = np.asarray(wo, dtype=np.float32)

    nc = _get_nc()
    in_maps = _host_inputs(x, cos, sin, wq, wk, wv, wo)
    res = run_bass_kernel_spmd(nc, in_maps, core_ids=list(range(8)), trace=trace)
    out = np.zeros((2, T, C), dtype=np.float32)
    for c in range(8):
        b = c // 4
        out[b] += res.results[c]["y"].astype(np.float32)
    if trace:
        return out, res
    return out


# revision 15
# speedup vs baseline: 1.2458x; 1.0027x over previous
"""Causal self-attention (RoPE + QK-RMSNorm, GQA 16q/8kv) Trainium2 Bass kernel.

Sharding: 8 cores = 2 batch x 4 tensor-parallel. Core c handles batch b=c//4 and
q-heads [4*tp, 4*tp+4), kv-heads [2*tp, 2*tp+2) where tp=c%4. Each core returns a
partial (T, C) output = O_heads @ wo[rows of its heads]; host sums the 4 partials
per batch (the "all-reduce after c_proj").

Matmuls run in bf16 (fp32 PSUM accumulation); softmax row-sum normalization and
RMS statistics stay in fp32/fp32r.

Phase-3 structure: scores for 2 key-blocks land in one 2-bank PSUM tile, one exp
instruction covers both; row-sums use a DVE pair-add + one accumulating matmul
per group; projection matmuls of the previous span interleave into the current
span's attention stream to keep the PE dense (HAM stays at full clock).
"""
import sys
import math

sys.path.insert(0, "/opt/trn_rl_repo")

import numpy as np
import ml_dtypes
import concourse.bacc as bacc
import concourse.mybir as mybir
import concourse.tile as tile
from concourse.bass_utils import run_bass_kernel_spmd

P = 128
T = 2048
C = 2048
KO = C // P          # 16 contraction tiles
D = 128              # head dim
NQ = 4               # q heads per core
NK = 2               # kv heads per core
NF = NQ + NK         # 6 rope/rms feature blocks (4 q + 2 k)
FQ = NQ * D          # 512
FK = NK * D          # 256
TCH = 512            # phase-1 T-chunk
NCHUNK = T // TCH    # 4
SPAN = 512           # attention q-span
NSPAN = T // SPAN    # 4
KB = T // P          # 16 key blocks
SCALE = 1.0 / math.sqrt(D)
EPS = 1.1920929e-07

f32 = mybir.dt.float32
f32r = mybir.dt.float32r
bf16 = mybir.dt.bfloat16

AF = mybir.ActivationFunctionType


def build():
    nc = bacc.Bacc("TRN2", target_bir_lowering=False)
    xT = nc.dram_tensor("xT", (C, T), bf16, kind="ExternalInput")
    wq = nc.dram_tensor("wq", (C, FQ), bf16, kind="ExternalInput")
    wk = nc.dram_tensor("wk", (C, FK), bf16, kind="ExternalInput")
    wv = nc.dram_tensor("wv", (C, FK), bf16, kind="ExternalInput")
    wo = nc.dram_tensor("wo", (FQ, C), bf16, kind="ExternalInput")
    cc = nc.dram_tensor("cc", (P, T), bf16, kind="ExternalInput")    # [cos; cos]
    ss = nc.dram_tensor("ss", (P, T), bf16, kind="ExternalInput")    # [sin; -sin]
    maskT = nc.dram_tensor("maskT", (P, 4, SPAN), bf16, kind="ExternalInput")
    y = nc.dram_tensor("y", (T, C), bf16, kind="ExternalOutput")

    xT_r = xT.rearrange("(ko p) t -> p ko t", p=P)
    wq_r = wq.rearrange("(ko p) f -> p ko f", p=P)
    wk_r = wk.rearrange("(ko p) f -> p ko f", p=P)
    wv_r = wv.rearrange("(ko p) f -> p ko f", p=P)
    wo_r = wo.rearrange("(ko p) n -> p ko n", p=P)

    with tile.TileContext(nc) as tc:
        with tc.tile_pool(name="persist", bufs=1) as persist:
            # persistent across phases
            qk_rt = persist.tile([P, NF, T], bf16, tag="qk_rt")   # roped+normed qT/kT
            v_sb = persist.tile([P, KB, FK], bf16, tag="v_sb")    # V natural [t-part, kb, feat]
            cc_sb = persist.tile([P, T], bf16, tag="cc_sb")
            ss_sb = persist.tile([P, T], bf16, tag="ss_sb")
            ones_col = persist.tile([P, 1], bf16, tag="ones_col")    # sums lhsT
            ones_row = persist.tile([1, P], f32r, tag="ones_row")    # bcast lhsT
            eps_sb = persist.tile([P, 1], f32, tag="eps_sb")
            zero_sb = persist.tile([1, 1], f32, tag="zero_sb")
            nc.vector.memset(zero_sb[:], 0.0)
            ones_f32 = persist.tile([P, 1], f32, tag="ones_f32")
            ones_row_f32 = persist.tile([1, P], f32, tag="ones_row_f32")
            nc.vector.memset(eps_sb[:], EPS)
            nc.vector.memset(ones_f32[:], 1.0)
            nc.vector.memset(ones_row_f32[:], 1.0)
            nc.vector.tensor_copy(ones_col[:], ones_f32[:])
            nc.vector.tensor_copy(ones_row[:], ones_row_f32[:])

            # ------- Phase 1: QKV projections + RoPE + RMS norm + V transpose -------
            with (
                tc.tile_pool(name="ph1w", bufs=1) as wpool,
                tc.tile_pool(name="ph1x", bufs=2) as xpool,
                tc.tile_pool(name="ph1t", bufs=3) as tpool,
                tc.tile_pool(name="ph1ps", bufs=4, space="PSUM") as ps1,
                tc.tile_pool(name="ph1ms", bufs=1, space="PSUM") as psms,
                tc.tile_pool(name="ph1rb", bufs=2, space="PSUM") as psrb,
            ):
                wq_sb = wpool.tile([P, KO, FQ], bf16, tag="wq_sb")
                wk_sb = wpool.tile([P, KO, FK], bf16, tag="wk_sb")
                wv_sb = wpool.tile([P, KO, FK], bf16, tag="wv_sb")
                # startup-critical DMA order: first chunk's x plus weights,
                # per-ko and interleaved in matmul consumption order (v first),
                # so the first matmul group can start after ~one ko-slice.
                xt0 = xpool.tile([P, KO, TCH], bf16, tag="xt", bufs=3)
                for ko in range(KO):
                    nc.sync.dma_start(xt0[:, ko, :], xT_r[:, ko, 0:TCH])
                    nc.sync.dma_start(wv_sb[:, ko, :], wv_r[:, ko, :])
                for ko in range(KO):
                    nc.sync.dma_start(wq_sb[:, ko, :], wq_r[:, ko, :])
                    if ko == 2:  # needed by the first rope post
                        nc.sync.dma_start(cc_sb[:], cc[:, :])
                        nc.sync.dma_start(ss_sb[:], ss[:, :])
                for ko in range(KO):
                    nc.sync.dma_start(wk_sb[:, ko, :], wk_r[:, ko, :])

                # Software-pipelined over feature blocks, carried across chunk
                # boundaries: each block's rope/RMS/transpose post-processing is
                # deferred by 2 matmul groups (~7us of PE cover) so the in-order
                # DVE queue and the swap DMAs never stall the PE. The RMS
                # broadcast+apply trails one further post-step.
                posts = []   # deferred post-process closures
                appls = []   # deferred RMS broadcast+apply

                def emit_post_qk(args):
                    fb, tch, raw, swp = args
                    t0 = tch * TCH
                    tmpa = tpool.tile([P, TCH], bf16, tag="rope_tmpa")
                    tmpb = tpool.tile([P, TCH], bf16, tag="rope_tmpb")
                    seg = qk_rt[:, fb, t0 : t0 + TCH]
                    nc.vector.tensor_mul(tmpa[:], raw[:], cc_sb[:, t0 : t0 + TCH])
                    nc.vector.tensor_mul(tmpb[:], swp[:], ss_sb[:, t0 : t0 + TCH])
                    nc.vector.tensor_add(seg, tmpa[:], tmpb[:])
                    # RMS stats: sum of squares over head dim (partitions)
                    sq = tpool.tile([P, TCH], bf16, tag="sq")
                    nc.vector.tensor_mul(sq[:], seg, seg)
                    pms = psms.tile([1, TCH], f32, tag="ps_ms")
                    nc.tensor.matmul(pms[:], ones_col[:], sq[:], start=True, stop=True)
                    # rstd = 1/sqrt(ms/D + eps): ACT Sqrt + fast DVE recip.
                    # (Ln+Exp here would thrash ACT table sets against the
                    # attention phase's Exp — 1.28us per switch.)
                    sqms = tpool.tile([1, TCH], f32, tag="sqms", bufs=5)
                    nc.scalar.activation(
                        sqms[:], pms[:], AF.Sqrt, bias=eps_sb[0:1, :], scale=1.0 / D
                    )
                    rstd_raw = tpool.tile([1, TCH], f32, tag="rstd_raw", bufs=5)
                    nc.vector.reciprocal_approx_fast(rstd_raw[:], sqms[:])
                    # f32r for a single-pass fp32 broadcast matmul (plain f32
                    # lhsT/rhs lowers to two half-rate MATMUL passes)
                    rstd = tpool.tile([1, TCH], f32r, tag="rstd", bufs=5)
                    nc.scalar.activation(rstd[:], rstd_raw[:], AF.Copy)
                    appls.append((seg, rstd))

                def emit_post_v(args):
                    vfb, tch, vt_sb = args
                    # hardware xbar transpose: v_sb[p, kb, f] = vt_sb[f, kb*128+p]
                    nc.sync.dma_start(
                        v_sb[
                            :,
                            tch * (TCH // P) : (tch + 1) * (TCH // P),
                            vfb * D : (vfb + 1) * D,
                        ],
                        vt_sb[:],
                        transpose=True,
                    )

                def drain_one_post():
                    kind, args = posts.pop(0)
                    (emit_post_qk if kind == "qk" else emit_post_v)(args)
                    if len(appls) > 1:
                        seg, rstd = appls.pop(0)
                        pb = psrb.tile([P, TCH], f32, tag="ps_b")
                        nc.tensor.matmul(
                            pb[:], ones_row[:], rstd[:], start=True, stop=True
                        )
                        nc.vector.tensor_mul(seg, seg, pb[:])

                for tch in range(NCHUNK):
                    t0 = tch * TCH
                    if tch == 0:
                        xt = xt0
                    else:
                        xt = xpool.tile([P, KO, TCH], bf16, tag="xt", bufs=3)
                        for ko in range(KO):
                            nc.sync.dma_start(xt[:, ko, :], xT_r[:, ko, t0 : t0 + TCH])
                    # v first: its post (PE transposes) is cheap and swap-free
                    for vfb in range(NK):
                        w_ap = wv_sb[:, :, vfb * D : (vfb + 1) * D]
                        pvt = ps1.tile([P, TCH], f32, tag="ps_qkv")
                        for ko in range(KO):
                            nc.tensor.matmul(
                                pvt[:], w_ap[:, ko], xt[:, ko, :],
                                start=(ko == 0), stop=(ko == KO - 1),
                            )
                        vt_sb = tpool.tile([P, TCH], bf16, tag="vt_sb", bufs=3)
                        nc.vector.tensor_copy(vt_sb[:], pvt[:])
                        posts.append(("v", (vfb, tch, vt_sb)))
                        if len(posts) > 2:
                            drain_one_post()
                    for fb in range(NF):
                        if fb < NQ:
                            w_ap = wq_sb[:, :, fb * D : (fb + 1) * D]
                        else:
                            w_ap = wk_sb[:, :, (fb - NQ) * D : (fb - NQ + 1) * D]
                        pqk = ps1.tile([P, TCH], f32, tag="ps_qkv")
                        for ko in range(KO):
                            nc.tensor.matmul(
                                pqk[:], w_ap[:, ko], xt[:, ko, :],
                                start=(ko == 0), stop=(ko == KO - 1),
                            )
                        raw = tpool.tile([P, TCH], bf16, tag="rope_raw", bufs=5)
                        nc.vector.tensor_copy(raw[:], pqk[:])
                        swp = tpool.tile([P, TCH], bf16, tag="rope_swp", bufs=5)
                        nc.sync.dma_start(swp[0:64, :], raw[64:128, :])
                        nc.sync.dma_start(swp[64:128, :], raw[0:64, :])
                        posts.append(("qk", (fb, tch, raw, swp)))
                        if len(posts) > 2:
                            drain_one_post()
                while posts:
                    drain_one_post()
                while appls:
                    seg, rstd = appls.pop(0)
                    pb = psrb.tile([P, TCH], f32, tag="ps_b")
                    nc.tensor.matmul(
                        pb[:], ones_row[:], rstd[:], start=True, stop=True
                    )
                    nc.vector.tensor_mul(seg, seg, pb[:])

            # ---------------- Phase 3: attention + Phase 4: output projection ------------
            with (
                tc.tile_pool(name="ph3s", bufs=1) as p3s,
                tc.tile_pool(name="ph3t", bufs=3) as p3,
                tc.tile_pool(name="ph3y", bufs=3) as p3y,
            ):
                ot_sb = p3s.tile([P, NQ, T], bf16, tag="ot_sb")
                mask_sb = p3s.tile([P, 4, SPAN], bf16, tag="mask_sb")
                wo_sb = p3s.tile([P, NQ, C], bf16, tag="wo_sb")
                nc.sync.dma_start(mask_sb[:], maskT[:, :, :])
                for ko in range(NQ):
                    nc.sync.dma_start(wo_sb[:, ko, :], wo_r[:, ko, :])

                with (
                    tc.tile_pool(name="ph3sc", bufs=2, space="PSUM") as ps_sc,
                    tc.tile_pool(name="ph3ot", bufs=1, space="PSUM") as ps_ot,
                    tc.tile_pool(name="ph3nm", bufs=1, space="PSUM") as ps_nm,
                    tc.tile_pool(name="ph3yp", bufs=2, space="PSUM") as ps_yp,
                ):
                    yi = 0  # global proj-group counter (for engine alternation)

                    # zero both score slots once: narrowed diagonal matmuls
                    # leave part of the bank untouched, and uninitialized PSUM
                    # could hold NaN garbage that exp would propagate
                    for _ in range(2):
                        scz = ps_sc.tile([P, 2, SPAN], f32, tag="sc")
                        nc.vector.memset(scz[:], 0.0)

                    def emit_proj_group(tb, nch):
                        nonlocal yi
                        yps = ps_yp.tile([P, 512], f32, tag="yps")
                        for hh in range(NQ):
                            nc.tensor.matmul(
                                yps[:],
                                ot_sb[:, hh, tb * P : (tb + 1) * P],
                                wo_sb[:, hh, nch * 512 : (nch + 1) * 512],
                                start=(hh == 0), stop=(hh == NQ - 1),
                            )
                        ysb = p3y.tile([P, 512], bf16, tag="ysb")
                        # alternate the PSUM->SBUF evacuation between DVE and ACT
                        if yi % 2 == 0:
                            nc.vector.tensor_copy(ysb[:], yps[:])
                        else:
                            nc.scalar.activation(ysb[:], yps[:], AF.Copy)
                        nc.sync.dma_start(
                            y[tb * P : (tb + 1) * P, nch * 512 : (nch + 1) * 512],
                            ysb[:],
                        )
                        yi += 1

                    # FIFO of the previous span's projection groups; spent as PE
                    # filler at the per-head stall points (softmax-normalize
                    # chain latency, exp pipeline refill).
                    proj_fifo = []

                    def proj_fill(n):
                        for _ in range(min(n, len(proj_fifo))):
                            tb, nch = proj_fifo.pop(0)
                            emit_proj_group(tb, nch)

                    for s in range(NSPAN):
                        q0 = s * SPAN
                        nkb = 4 * s + 4
                        ng = nkb // 2       # score/exp groups of 2 key-blocks
                        for h in range(NQ):
                            j = h // 2
                            ot_ps = ps_ot.tile([P, SPAN], f32, tag="ot_ps")
                            sum_ps = ps_nm.tile([1, SPAN], f32, tag="nm")
                            q_ap = qk_rt[:, h, q0 : q0 + SPAN]
                            for g in range(ng):
                                sc = ps_sc.tile([P, 2, SPAN], f32, tag="sc")
                                for jj in range(2):
                                    kb = 2 * g + jj
                                    # diagonal blocks: only queries >= kb*128
                                    # are unmasked — skip the dead columns
                                    qo = max(0, (kb - 4 * s) * P)
                                    nc.tensor.matmul(
                                        sc[:, jj, qo:],
                                        qk_rt[:, NQ + j, kb * P : (kb + 1) * P],
                                        q_ap[:, qo:],
                                        start=True, stop=True,
                                    )
                                pt = p3.tile([P, 2, SPAN], bf16, tag="pt")
                                nc.scalar.activation(
                                    pt[:, :, :], sc[:, :, :], AF.Exp, scale=SCALE
                                )
                                if g == 0:
                                    # cover the exp latency before the first AV
                                    proj_fill(2)
                                if g >= 2 * s:  # diagonal groups: causal mask
                                    r = 2 * g - 4 * s
                                    nc.vector.tensor_mul(
                                        pt[:, :, :], pt[:, :, :],
                                        mask_sb[:, r : r + 2, :],
                                    )
                                # AV accumulation
                                for jj in range(2):
                                    kb = 2 * g + jj
                                    qo = max(0, (kb - 4 * s) * P)
                                    nc.tensor.matmul(
                                        ot_ps[:, qo:],
                                        v_sb[:, kb, j * D : (j + 1) * D],
                                        pt[:, jj, qo:],
                                        start=(kb == 0), stop=(kb == nkb - 1),
                                        skip_group_check=True,
                                    )
                                # row-sum: pair-add on DVE, one matmul per group
                                lf = p3.tile([P, SPAN], bf16, tag="lf")
                                nc.vector.tensor_add(lf[:], pt[:, 0, :], pt[:, 1, :])
                                nc.tensor.matmul(
                                    sum_ps[:], ones_col[:], lf[:],
                                    start=(g == 0), stop=(g == ng - 1),
                                    skip_group_check=True,
                                )
                            # normalization: fast 1/sums + f32r copy on DVE (off
                            # the ACT exp queue), broadcast via PE
                            rec_raw = p3.tile([1, SPAN], f32, tag="rec_raw")
                            nc.vector.reciprocal_approx_fast(rec_raw[:], sum_ps[:])
                            rec = p3.tile([1, SPAN], f32r, tag="rec")
                            nc.vector.tensor_copy(rec[:], rec_raw[:])
                            proj_fill(1)  # cover the reciprocal chain latency
                            bc_ps = ps_nm.tile([P, SPAN], f32, tag="nm")
                            nc.tensor.matmul(
                                bc_ps[:], ones_row[:], rec[:],
                                start=True, stop=True,
                            )
                            bc_sb = p3.tile([P, SPAN], f32, tag="bc_sb")
                            nc.vector.tensor_copy(bc_sb[:], bc_ps[:])
                            nc.vector.tensor_mul(
                                ot_sb[:, h, q0 : q0 + SPAN], ot_ps[:], bc_sb[:]
                            )
                            proj_fill(1)
                        # refill the FIFO with this span's projection work for
                        # the next span (plus any leftovers)
                        for tb in range(4 * s, 4 * s + 4):
                            for nch in range(C // 512):
                                proj_fifo.append((tb, nch))

                    # tail: drain the last span's projection
                    while proj_fifo:
                        tb, nch = proj_fifo.pop(0)
                        emit_proj_group(tb, nch)
    nc.compile()
    return nc


_NC_CACHE = None


def _get_nc():
    global _NC_CACHE
    if _NC_CACHE is None:
        _NC_CACHE = build()
    return _NC_CACHE


def _host_inputs(x, cos, sin, wq, wk, wv, wo):
    """Build the 8 per-core input maps."""
    bft = ml_dtypes.bfloat16
    cosT = np.ascontiguousarray(cos[0, :, 0, :].T).astype(np.float32)  # (64, T)
    sinT = np.ascontiguousarray(sin[0, :, 0, :].T).astype(np.float32)
    cc = np.concatenate([cosT, cosT], axis=0).astype(bft)          # (128, T)
    ss = np.concatenate([sinT, -sinT], axis=0).astype(bft)
    # maskT[r][k, q] = 1 if q >= 128*r + k  (within a 512-q span, k-block offset r)
    qidx = np.arange(SPAN)[None, None, :]
    kidx = np.arange(P)[:, None, None]
    ridx = np.arange(4)[None, :, None]
    maskT = (qidx >= P * ridx + kidx).astype(bft)  # (128, 4, 512)
    ident = np.eye(P, dtype=np.float32).astype(bft)

    xTs = [np.ascontiguousarray(x[b].T).astype(bft) for b in range(2)]
    wq16 = wq.astype(bft)
    wk16 = wk.astype(bft)
    wv16 = wv.astype(bft)
    wo16 = wo.astype(bft)
    in_maps = []
    for c in range(8):
        b, tp = divmod(c, 4)
        in_maps.append(
            {
                "xT": xTs[b],
                "wq": np.ascontiguousarray(wq16[:, tp * FQ : (tp + 1) * FQ]),
                "wk": np.ascontiguousarray(wk16[:, tp * FK : (tp + 1) * FK]),
                "wv": np.ascontiguousarray(wv16[:, tp * FK : (tp + 1) * FK]),
                "wo": np.ascontiguousarray(wo16[tp * FQ : (tp + 1) * FQ, :]),
                "cc": cc,
                "ss": ss,
                "maskT": maskT,
            }
        )
    return in_maps


def kernel(x, cos, sin, wq, wk, wv, wo, trace=False):
    x = np.asarray(x, dtype=np.float32)
    cos = np.asarray(cos, dtype=np.float32)
    sin = np.asarray(sin, dtype=np.float32)
    wq = np.asarray(wq, dtype=np.float32)
    wk = np.asarray(wk, dtype=np.float32)
    wv = np.asarray(wv, dtype=np.float32)
    wo = np.asarray(wo, dtype=np.float32)

    nc = _get_nc()
    in_maps = _host_inputs(x, cos, sin, wq, wk, wv, wo)
    res = run_bass_kernel_spmd(nc, in_maps, core_ids=list(range(8)), trace=trace)
    out = np.zeros((2, T, C), dtype=np.float32)
    for c in range(8):
        b = c // 4
        out[b] += res.results[c]["y"].astype(np.float32)
    if trace:
        return out, res
    return out
